# revision 11
# baseline (speedup 1.0000x reference)
"""4-layer GCN (N=50000, E=1.6M, F=128) on 8 Trainium2 NeuronCores.

Strategy:
  - Destination-node sharding: core c owns nodes [c*6250, (c+1)*6250).
  - Per layer: each core computes xW for its node shard (TensorE), shards are
    AllGathered into a full HBM feature table [50176, 128] fp16.
  - Message passing: edges (sorted by dst window) are gathered from the table
    via GPSIMD dma_gather (one 256B descriptor per edge); the weighted
    segment-sum over destinations is computed as mask matmuls on TensorE:
        mask[e, d] = norm_e * (dst_e == d)        (one DVE tensor_scalar op)
        psum[f, d] += gathered[e, f].T @ mask[e, d]   (fp32 PSUM accumulation)
  - Edges are split into two halves by source table row (int16 gather index
    limit); half A accumulates into PSUM and is staged to SBUF f32 (with the
    fused BN scale/bias), half B accumulates in PSUM and is combined with the
    staged value on DVE, then ReLU'd on ScalarE.
  - BatchNorms (eval mode) are folded into per-feature scale/bias applied on
    the PSUM->SBUF path. The final MLP head runs on-chip as well.

Host/transfer path (the wall-clock bottleneck on axon-tunneled cores):
  - The jitted PJRT executable is cached across kernel() calls (the stock
    run_bass_kernel_spmd re-traces and re-compiles the XLA wrapper per call).
  - Uploads are minimized: x ships as f8e4m3 (transposed on device via
    identity matmuls), gather indices ship unreplicated and are replicated
    16->128 partitions on device, dst-window as u8, edge norms as f16.
  - The kernel fully writes its output, so the pre-zeroed output operands
    the PJRT wrapper wants are never read: one persistent device-resident
    dummy is reused (no donation), saving a ~70ms dispatch round trip/call.
  - Output returns u8-quantized with per-feature min/max (computed on
    device) packed into the last 8 columns; the host dequantizes.
"""

import numpy as np
import ml_dtypes

N, E, F, C = 50000, 1600000, 128, 40
NCORES = 8
SH = N // NCORES            # 6250 nodes per core
NW = (SH + 127) // 128      # 49 dst windows per core
SHP = NW * 128              # 6272 padded shard rows
NP = NCORES * SHP           # 50176 padded table rows
HALF = NP // 2              # 25088 (int16-safe gather index range)
NG = NCORES * 2 * NW        # scatter groups
BN_EPS = 1e-5
G = 8                       # chunks (of 128 edges) per dma_gather call
                            # (SWDGE ring caps dma_gather at 1024 idxs/call)
WPK = 744                   # wpack cols: W1..W4 | iota | lw1 | lw2
F8 = ml_dtypes.float8_e4m3

_cache = {}
_shard_cache = []


def _shard():
    if not _shard_cache:
        import jax
        from jax.sharding import Mesh, PartitionSpec, NamedSharding
        devices = jax.devices()[:NCORES]
        mesh = Mesh(np.asarray(devices), ("core",))
        _shard_cache.append(NamedSharding(mesh, PartitionSpec("core")))
    return _shard_cache[0]


def _build_program(CC):
    """Build + compile the SPMD bass program. CC: [2][NW] chunks per
    (source-half, dst-window); identical across cores."""
    from concourse import bacc, tile, mybir, library_config

    FOUT = [128, 128, 64, 32]
    FIN = [128, 128, 128, 64]
    f32, f16 = mybir.dt.float32, mybir.dt.float16
    i16, u8, f8 = mybir.dt.int16, mybir.dt.uint8, mybir.dt.float8e4

    n_chunks = int(CC.sum())
    NIDX = n_chunks * 128

    nc = bacc.Bacc("TRN2", target_bir_lowering=False, debug=False,
                   num_devices=NCORES)

    # --- dram parameters ---
    x_d = nc.dram_tensor("x", [SHP, 128], f8, kind="ExternalInput")
    idx_d = nc.dram_tensor("idx", [16, NIDX // 16], i16, kind="ExternalInput")
    d8_d = nc.dram_tensor("d8", [128, n_chunks], u8, kind="ExternalInput")
    nrmh_d = nc.dram_tensor("nrm", [128, n_chunks], f16, kind="ExternalInput")
    sc_d = nc.dram_tensor("sc", [128, 14], f32, kind="ExternalInput")
    wpk_d = nc.dram_tensor("wpk", [128, WPK], f16, kind="ExternalInput")
    # u8-quantized logits; cols SHP..SHP+8 hold per-feature min/max f32 bytes
    out_d = nc.dram_tensor("out", [40, SHP + 8], u8, kind="ExternalOutput")

    shard_d = [nc.dram_tensor(f"shard{l}", [SHP, 128], f16) for l in range(4)]
    table_d = [nc.dram_tensor(f"table{l}", [NP, 128], f16, addr_space="Shared")
               for l in range(4)]

    with tile.TileContext(nc) as tc:
        nc.gpsimd.load_library(library_config.mlp)
        with tc.tile_pool(name="pers", bufs=1) as pers, \
             tc.tile_pool(name="hpool", bufs=2) as hpool, \
             tc.tile_pool(name="zpool", bufs=2) as zpool, \
             tc.tile_pool(name="gpool", bufs=2) as gpool, \
             tc.tile_pool(name="mpool", bufs=6) as mpool, \
             tc.tile_pool(name="tpool", bufs=4) as tpool, \
             tc.tile_pool(name="wpool", bufs=3) as wpool, \
             tc.tile_pool(name="opool", bufs=2) as opool, \
             tc.tile_pool(name="ppa", bufs=3, space="PSUM") as ppa, \
             tc.tile_pool(name="ppw", bufs=2, space="PSUM") as ppw, \
             tc.tile_pool(name="pph1", bufs=1, space="PSUM") as pph1, \
             tc.tile_pool(name="pph2", bufs=1, space="PSUM") as pph2:

            # --- persistent loads ---
            idx_t = pers.tile([128, NIDX // 16], i16)
            for k in range(8):   # replicate [16, L] -> 8x in 128 partitions
                nc.sync.dma_start(out=idx_t[16 * k:16 * (k + 1), :],
                                  in_=idx_d[:])
            d8_t = pers.tile([128, n_chunks], u8)
            nc.sync.dma_start(out=d8_t[:], in_=d8_d[:])
            dst_t = pers.tile([128, n_chunks], f32)
            nc.scalar.copy(out=dst_t[:], in_=d8_t[:])
            nrmh_t = pers.tile([128, n_chunks], f16)
            nc.sync.dma_start(out=nrmh_t[:], in_=nrmh_d[:])
            nrm_t = pers.tile([128, n_chunks], f32)
            nc.scalar.copy(out=nrm_t[:], in_=nrmh_t[:])
            sc_t = pers.tile([128, 14], f32)
            nc.sync.dma_start(out=sc_t[:], in_=sc_d[:])
            wpk_t = pers.tile([128, WPK], f16)
            nc.sync.dma_start(out=wpk_t[:], in_=wpk_d[:])
            W_t = [wpk_t[0:128, l * 128:(l + 1) * 128] for l in range(4)]
            iota_t = wpk_t[0:128, 512:640]
            lw1_t = wpk_t[0:32, 640:704]
            lw2_t = wpk_t[0:64, 704:744]
            # identity (f8) for the PE x-transpose: id[p, j] = (j == p)
            id8_t = pers.tile([128, 128], f8)
            nc.vector.tensor_scalar(
                id8_t[:], iota_t, sc_t[:, 13:14], None,
                mybir.AluOpType.is_equal)

            # --- x load (f8, [node, feat]) + PE transpose + BN1 ---
            xs_t = pers.tile([128, NW, 128], f8)
            for t in range(NW):
                nc.sync.dma_start(out=xs_t[:, t, :],
                                  in_=x_d[t * 128:(t + 1) * 128, :])
            hprev = hpool.tile([128, SHP], f16, tag="h")
            for t in range(NW):
                pw = ppw.tile([128, 128], mybir.dt.float32, space="PSUM",
                              tag="pw")
                nc.tensor.matmul(out=pw[:], lhsT=xs_t[:, t, :], rhs=id8_t[:],
                                 start=True, stop=True)
                nc.scalar.activation(
                    out=hprev[:, t * 128:(t + 1) * 128], in_=pw[:],
                    func=mybir.ActivationFunctionType.Identity,
                    bias=sc_t[:, 1:2], scale=sc_t[:, 0:1])

            # window -> chunk index ranges per half
            half_windows = []   # [2][NW] -> (chunk_lo, chunk_hi) global ids
            cg = 0
            for h in range(2):
                rngs = []
                for w in range(NW):
                    rngs.append((cg, cg + int(CC[h][w])))
                    cg += int(CC[h][w])
                half_windows.append(rngs)

            for l in range(4):
                fin, fout = FIN[l], FOUT[l]
                # --- xW pass over own shard ---
                for t in range(NW):
                    pw = ppw.tile([128, 128], mybir.dt.float32, space="PSUM",
                                  tag="pw")
                    nc.tensor.matmul(
                        out=pw[0:128, 0:fout],
                        lhsT=hprev[0:fin, t * 128:(t + 1) * 128],
                        rhs=W_t[l][0:fin, 0:fout],
                        start=True, stop=True)
                    xw = wpool.tile([128, 128], f16, tag="xw")
                    nc.scalar.activation(
                        out=xw[0:128, 0:fout], in_=pw[0:128, 0:fout],
                        func=mybir.ActivationFunctionType.Copy)
                    nc.sync.dma_start(
                        out=shard_d[l][t * 128:(t + 1) * 128, 0:fout],
                        in_=xw[0:128, 0:fout])
                nc.gpsimd.collective_compute(
                    "AllGather", mybir.AluOpType.bypass,
                    replica_groups=[list(range(NCORES))],
                    ins=[shard_d[l][:]], outs=[table_d[l][:]])

                # --- aggregation pass ---
                z_t = zpool.tile([128, SHP], mybir.dt.float32, tag="z")
                hnext = hpool.tile([128, SHP], f16, tag="h")
                a_ap = sc_t[0:fout, 2 + 2 * l:3 + 2 * l]
                b_ap = sc_t[0:fout, 3 + 2 * l:4 + 2 * l]
                for h in range(2):
                    rngs = half_windows[h]
                    c_lo, c_hi = rngs[0][0], rngs[-1][1]
                    tab_ap = table_d[l][h * HALF:(h + 1) * HALF, :]
                    # super-gather batches
                    pa = None
                    w_idx = 0  # current window
                    for s0 in range(c_lo, c_hi, G):
                        s1 = min(s0 + G, c_hi)
                        ncur = s1 - s0
                        gt = gpool.tile([128, G, 128], f16, tag="g")
                        nc.gpsimd.dma_gather(
                            out_ap=gt[:, 0:ncur, :], in_ap=tab_ap,
                            idxs_ap=idx_t[:, s0 * 8:s1 * 8],
                            num_idxs=ncur * 128, num_idxs_reg=ncur * 128,
                            elem_size=128)
                        for j in range(ncur):
                            cgi = s0 + j
                            while cgi >= rngs[w_idx][1]:
                                w_idx += 1
                            w_lo, w_hi = rngs[w_idx]
                            mask = mpool.tile([128, 128], f16, tag="m")
                            nc.vector.tensor_scalar(
                                mask[:], iota_t,
                                dst_t[:, cgi:cgi + 1], nrm_t[:, cgi:cgi + 1],
                                mybir.AluOpType.is_equal,
                                mybir.AluOpType.mult)
                            if cgi == w_lo:
                                pa = ppa.tile([128, 128], mybir.dt.float32,
                                              space="PSUM", tag="pa")
                            nc.tensor.matmul(
                                out=pa[0:fout, :],
                                lhsT=gt[:, j, 0:fout], rhs=mask[:],
                                start=(cgi == w_lo), stop=(cgi == w_hi - 1))
                            if cgi == w_hi - 1:
                                wsl = slice(w_idx * 128, (w_idx + 1) * 128)
                                if h == 0:
                                    # stage a*psumA + b into f32 z
                                    nc.scalar.activation(
                                        out=z_t[0:fout, wsl],
                                        in_=pa[0:fout, :],
                                        func=mybir.ActivationFunctionType.Identity,
                                        bias=b_ap, scale=a_ap)
                                else:
                                    # combine + relu
                                    tmp = tpool.tile([128, 128], f16, tag="t")
                                    nc.vector.scalar_tensor_tensor(
                                        out=tmp[0:fout, :],
                                        in0=pa[0:fout, :], scalar=a_ap,
                                        in1=z_t[0:fout, wsl],
                                        op0=mybir.AluOpType.mult,
                                        op1=mybir.AluOpType.add)
                                    nc.scalar.activation(
                                        out=hnext[0:fout, wsl],
                                        in_=tmp[0:fout, :],
                                        func=mybir.ActivationFunctionType.Relu)
                hprev = hnext

            # --- head: relu already applied to hprev (=relu(bn5(agg4))) ---
            a6_ap = sc_t[0:64, 10:11]
            b6_ap = sc_t[0:64, 11:12]
            lb2_ap = sc_t[0:40, 12:13]
            hq = pers.tile([40, SHP], f16)
            for t0 in range(0, SHP, 512):
                L = min(512, SHP - t0)
                p1 = pph1.tile([64, 512], mybir.dt.float32, space="PSUM",
                               tag="p1")
                nc.tensor.matmul(out=p1[0:64, 0:L], lhsT=lw1_t,
                                 rhs=hprev[0:32, t0:t0 + L],
                                 start=True, stop=True)
                h5 = opool.tile([64, 512], f16, tag="h5")
                nc.scalar.activation(out=h5[0:64, 0:L], in_=p1[0:64, 0:L],
                                     func=mybir.ActivationFunctionType.Relu,
                                     bias=b6_ap, scale=a6_ap)
                p2 = pph2.tile([40, 512], mybir.dt.float32, space="PSUM",
                               tag="p2")
                nc.tensor.matmul(out=p2[0:40, 0:L], lhsT=lw2_t,
                                 rhs=h5[0:64, 0:L], start=True, stop=True)
                nc.scalar.activation(out=hq[0:40, t0:t0 + L],
                                     in_=p2[0:40, 0:L],
                                     func=mybir.ActivationFunctionType.Identity,
                                     bias=lb2_ap)

            # --- quantize logits to u8 with per-feature min/max ---
            mx_t = pers.tile([40, 1], f32)
            mn_t = pers.tile([40, 1], f32)
            nc.vector.tensor_reduce(out=mx_t[:], in_=hq[0:40, 0:SH],
                                    axis=mybir.AxisListType.X,
                                    op=mybir.AluOpType.max)
            nc.vector.tensor_reduce(out=mn_t[:], in_=hq[0:40, 0:SH],
                                    axis=mybir.AxisListType.X,
                                    op=mybir.AluOpType.min)
            d_t = pers.tile([40, 1], f32)
            nc.vector.tensor_tensor(d_t[:], mx_t[:], mn_t[:],
                                    mybir.AluOpType.subtract)
            r_t = pers.tile([40, 1], f32)
            nc.vector.reciprocal(out=r_t[:], in_=d_t[:])
            s_t = pers.tile([40, 1], f32)   # 255/(max-min)
            nc.vector.tensor_scalar_mul(s_t[:], r_t[:], 255.0)
            nb_t = pers.tile([40, 1], f32)  # -min*255/(max-min)
            nc.vector.scalar_tensor_tensor(
                out=nb_t[:], in0=mn_t[:], scalar=-1.0, in1=s_t[:],
                op0=mybir.AluOpType.mult, op1=mybir.AluOpType.mult)
            for t0 in range(0, SHP, 1568):
                L = min(1568, SHP - t0)
                q = opool.tile([40, 1568], u8, tag="q")
                nc.scalar.activation(out=q[0:40, 0:L],
                                     in_=hq[0:40, t0:t0 + L],
                                     func=mybir.ActivationFunctionType.Identity,
                                     bias=nb_t[:], scale=s_t[:])
                nc.sync.dma_start(out=out_d[0:40, t0:t0 + L],
                                  in_=q[0:40, 0:L])
            nc.sync.dma_start(out=out_d[0:40, SHP:SHP + 4],
                              in_=mn_t[:].bitcast(u8))
            nc.sync.dma_start(out=out_d[0:40, SHP + 4:SHP + 8],
                              in_=mx_t[:].bitcast(u8))
    nc.compile()
    return nc


class _Runner:
    """Caches the jitted PJRT executable for one compiled bass program."""

    def __init__(self, nc):
        import jax
        from jax.sharding import PartitionSpec
        from jax.experimental.shard_map import shard_map
        from concourse import mybir, bass2jax
        from concourse.bass2jax import _bass_exec_p, partition_id_tensor

        bass2jax.install_neuronx_cc_hook()
        self.jax = jax
        self.nc = nc
        self.shard = _shard()

        partition_name = (nc.partition_id_tensor.name
                          if nc.partition_id_tensor else None)
        in_names, out_names, out_avals, out_shapes = [], [], [], []
        for alloc in nc.m.functions[0].allocations:
            if not isinstance(alloc, mybir.MemoryLocationSet):
                continue
            name = alloc.memorylocations[0].name
            if alloc.kind == "ExternalInput":
                if name != partition_name:
                    in_names.append(name)
            elif alloc.kind == "ExternalOutput":
                shape = tuple(alloc.tensor_shape)
                dtype = mybir.dt.np(alloc.dtype)
                out_names.append(name)
                out_avals.append(jax.core.ShapedArray(shape, dtype))
                out_shapes.append((shape, dtype))
        self.in_names = list(in_names)
        self.out_names = list(out_names)
        n_params = len(in_names)
        n_outs = len(out_names)
        all_in = in_names + out_names + (
            [partition_name] if partition_name else [])

        def _body(*args):
            operands = list(args)
            if partition_name is not None:
                operands.append(partition_id_tensor())
            outs = _bass_exec_p.bind(
                *operands, out_avals=tuple(out_avals),
                in_names=tuple(all_in), out_names=tuple(out_names),
                lowering_input_output_aliases=(),
                sim_require_finite=True, sim_require_nnan=True, nc=nc)
            return tuple(outs)

        mesh = self.shard.mesh
        in_specs = (PartitionSpec("core"),) * (n_params + n_outs)
        out_specs = (PartitionSpec("core"),) * n_outs
        # The kernel fully writes its outputs, so the "pre-zeroed output"
        # operands are never read on device: skip donation and reuse one
        # persistent dummy buffer per output across calls.
        self.fn = jax.jit(
            shard_map(_body, mesh=mesh, in_specs=in_specs,
                      out_specs=out_specs, check_rep=False),
            keep_unused=True)
        self.z = tuple(
            jax.device_put(np.zeros((NCORES * s[0], *s[1:]), dt), self.shard)
            for s, dt in out_shapes)

    def __call__(self, arrays):
        args = [arrays[nm] for nm in self.in_names]
        outs = self.fn(*args, *self.z)
        return dict(zip(self.out_names, outs))

    def put(self, a):
        return self.jax.device_put(a, self.shard)


def _prep_edges(inputs):
    """Edge partitioning, normalization, packing. ~0.2s of numpy."""
    ei = np.asarray(inputs["edge_index"])
    ew = np.asarray(inputs["edge_weight"], np.float32)
    M = E + N
    loop = np.arange(N, dtype=np.int32)
    src = np.empty(M, np.int32)
    src[:E] = ei[0]
    src[E:] = loop
    dst = np.empty(M, np.int32)
    dst[:E] = ei[1]
    dst[E:] = loop
    w = np.empty(M, np.float32)
    w[:E] = ew
    w[E:] = 1.0

    deg = np.bincount(dst, weights=w, minlength=N)
    dinv = (1.0 / np.sqrt(np.maximum(deg, 1e-12))).astype(np.float32)
    norm = dinv[src] * w * dinv[dst]

    core, dstl = np.divmod(dst, SH)
    win = dstl >> 7
    csrc, lsrc = np.divmod(src, SH)
    half = (csrc >= NCORES // 2).astype(np.int32)
    gid = ((core * 2 + half) * NW + win).astype(np.int16)  # < NG = 784
    order = np.argsort(gid, kind="stable")  # radix sort on int16

    gsz = np.bincount(gid, minlength=NG)
    CC = np.maximum(
        (gsz.reshape(NCORES, 2, NW).max(axis=0) + 127) // 128, 1)  # [2, NW]
    n_chunks = int(CC.sum())
    NIDX = n_chunks * 128

    # adj[g] = flat scatter position of group g's first element minus its
    # sorted start, so that flat_pos = arange + adj[gid_sorted]
    pad = (CC.astype(np.int64) * 128).ravel()                # [2*NW]
    off = np.zeros(2 * NW, np.int64)
    off[1:] = np.cumsum(pad)[:-1]
    gstart = np.zeros(NG, np.int64)
    gstart[1:] = np.cumsum(gsz)[:-1]
    coreg = np.arange(NG, dtype=np.int64) // (2 * NW)
    adj = (coreg * NIDX + np.tile(off, NCORES) - gstart).astype(np.int64)
    flat_pos = np.arange(M, dtype=np.int64) + adj[gid[order]]

    srow16 = ((csrc - (NCORES // 2) * half) * SHP + lsrc).astype(np.int16)
    dwin8 = (dstl & 127).astype(np.uint8)
    nrm16 = norm.astype(np.float16)

    IDX = np.zeros(NCORES * NIDX, np.int16)
    D8 = np.zeros(NCORES * NIDX, np.uint8)
    NRM = np.zeros(NCORES * NIDX, np.float16)
    IDX[flat_pos] = srow16[order]
    D8[flat_pos] = dwin8[order]
    NRM[flat_pos] = nrm16[order]
    IDX = IDX.reshape(NCORES, NIDX)
    D8 = D8.reshape(NCORES, NIDX)
    NRM = NRM.reshape(NCORES, NIDX)

    idx_g = np.empty((NCORES * 16, NIDX // 16), np.int16)
    d8_g = np.empty((NCORES * 128, n_chunks), np.uint8)
    nrm_g = np.empty((NCORES * 128, n_chunks), np.float16)
    for c in range(NCORES):
        idx_g[c * 16:(c + 1) * 16] = IDX[c].reshape(-1, 16).T
        d8_g[c * 128:(c + 1) * 128] = D8[c].reshape(-1, 128).T
        nrm_g[c * 128:(c + 1) * 128] = NRM[c].reshape(-1, 128).T
    return CC, {"idx": idx_g, "d8": d8_g, "nrm": nrm_g}


def _prep_weights(inputs):
    """Small replicated tensors: packed weights + folded BN scale/bias."""
    FIN = [128, 128, 128, 64]
    FOUT = [128, 128, 64, 32]
    wpk = np.zeros((128, WPK), np.float16)
    for l in range(4):
        Wl = np.asarray(inputs[f"W{l+1}"], np.float32)
        wpk[:FIN[l], l * 128:l * 128 + FOUT[l]] = Wl.astype(np.float16)
    wpk[:, 512:640] = np.arange(128, dtype=np.float16)[None, :]
    wpk[:32, 640:704] = np.asarray(inputs["lw1"], np.float32).astype(np.float16)
    wpk[:64, 704:744] = np.asarray(inputs["lw2"], np.float32).astype(np.float16)

    sc = np.zeros((128, 14), np.float32)
    g1 = np.asarray(inputs["g1"], np.float32)
    s1 = g1 / np.sqrt(np.asarray(inputs["var1"], np.float32) + BN_EPS)
    sc[:, 0] = s1
    sc[:, 1] = np.asarray(inputs["beta1"], np.float32) - \
        np.asarray(inputs["mu1"], np.float32) * s1
    for l in range(4):
        bn = l + 2
        gl = np.asarray(inputs[f"g{bn}"], np.float32)
        a = gl / np.sqrt(np.asarray(inputs[f"var{bn}"], np.float32) + BN_EPS)
        b = (np.asarray(inputs[f"c{l+1}b"], np.float32) -
             np.asarray(inputs[f"mu{bn}"], np.float32)) * a + \
            np.asarray(inputs[f"beta{bn}"], np.float32)
        sc[:FOUT[l], 2 + 2 * l] = a
        sc[:FOUT[l], 3 + 2 * l] = b
    a6 = np.asarray(inputs["g6"], np.float32) / \
        np.sqrt(np.asarray(inputs["var6"], np.float32) + BN_EPS)
    b6 = (np.asarray(inputs["lb1"], np.float32) -
          np.asarray(inputs["mu6"], np.float32)) * a6 + \
        np.asarray(inputs["beta6"], np.float32)
    sc[:64, 10] = a6
    sc[:64, 11] = b6
    sc[:40, 12] = np.asarray(inputs["lb2"], np.float32)
    sc[:, 13] = np.arange(128, dtype=np.float32)

    scg = np.empty((NCORES * 128, 14), np.float32)
    wpkg = np.empty((NCORES * 128, WPK), np.float16)
    for c in range(NCORES):
        scg[c * 128:(c + 1) * 128] = sc
        wpkg[c * 128:(c + 1) * 128] = wpk
    return {"sc": scg, "wpk": wpkg}


def kernel(**inputs):
    import jax

    # cast + pad x to f8 and dispatch its upload first: the transfer
    # streams in the background while the edge preprocessing runs
    x = np.asarray(inputs["x"], np.float32)
    xp = np.zeros((NCORES * SHP, 128), F8)
    for c in range(NCORES):
        xp[c * SHP:c * SHP + SH] = x[c * SH:(c + 1) * SH]
    shard = _shard()
    arrays = {"x": jax.device_put(xp, shard)}
    for k, v in _prep_weights(inputs).items():
        arrays[k] = jax.device_put(v, shard)

    CC, edict = _prep_edges(inputs)
    key = CC.tobytes()
    if key not in _cache:
        nc = _build_program(CC)
        _cache[key] = _Runner(nc)
    run = _cache[key]
    for k, v in edict.items():
        arrays[k] = run.put(v)
    res = run(arrays)

    out_h = np.asarray(res["out"])     # [NCORES*40, SHP+8] u8
    out = np.empty((N, C), np.float32)
    for c in range(NCORES):
        blk = out_h[c * 40:(c + 1) * 40]
        mn = blk[:, SHP:SHP + 4].copy().view(np.float32)[:, 0]
        mx = blk[:, SHP + 4:SHP + 8].copy().view(np.float32)[:, 0]
        s = (mx - mn) * np.float32(1.0 / 255.0)
        out[c * SH:(c + 1) * SH] = \
            (blk[:, :SH] * s[:, None] + mn[:, None]).T
    return out


# revision 14
# speedup vs baseline: 1.0723x; 1.0723x over previous
"""4-layer GCN (N=50000, E=1.6M, F=128) on 8 Trainium2 NeuronCores.

Strategy:
  - Destination-node sharding: core c owns nodes [c*6250, (c+1)*6250).
  - Per layer: each core computes xW for its node shard (TensorE), shards are
    AllGathered into a full HBM feature table [50176, 128] fp16.
  - Message passing: edges (sorted by dst window) are gathered from the table
    via GPSIMD dma_gather (one 256B descriptor per edge); the weighted
    segment-sum over destinations is computed as mask matmuls on TensorE:
        mask[e, d] = norm_e * (dst_e == d)        (one DVE tensor_scalar op)
        psum[f, d] += gathered[e, f].T @ mask[e, d]   (fp32 PSUM accumulation)
  - Edges are split into two halves by source table row (int16 gather index
    limit); half A accumulates into PSUM and is staged to SBUF f32 (with the
    fused BN scale/bias), half B accumulates in PSUM and is combined with the
    staged value on DVE, then ReLU'd on ScalarE.
  - BatchNorms (eval mode) are folded into per-feature scale/bias applied on
    the PSUM->SBUF path. The final MLP head runs on-chip as well.

Host/transfer path (the wall-clock bottleneck on axon-tunneled cores):
  - The jitted PJRT executable is cached across kernel() calls (the stock
    run_bass_kernel_spmd re-traces and re-compiles the XLA wrapper per call).
  - Uploads are minimized: x ships as f8e4m3 (transposed on device via
    identity matmuls), gather indices ship unreplicated and are replicated
    16->128 partitions on device, dst-window as u8, edge norms as f16.
  - The kernel fully writes its output, so the pre-zeroed output operands
    the PJRT wrapper wants are never read: one persistent device-resident
    dummy is reused (no donation), saving a ~70ms dispatch round trip/call.
  - Output returns u8-quantized with per-feature min/max (computed on
    device) packed into the last 8 columns; the host dequantizes.
"""

import numpy as np
import ml_dtypes

N, E, F, C = 50000, 1600000, 128, 40
NCORES = 8
SH = N // NCORES            # 6250 nodes per core
NW = (SH + 127) // 128      # 49 dst windows per core
SHP = NW * 128              # 6272 padded shard rows
NP = NCORES * SHP           # 50176 padded table rows
HALF = NP // 2              # 25088 (int16-safe gather index range)
NG = NCORES * 2 * NW        # scatter groups
BN_EPS = 1e-5
G = 8                       # chunks (of 128 edges) per dma_gather call
                            # (SWDGE ring caps dma_gather at 1024 idxs/call)
WPK = 744                   # wpack cols: W1..W4 | iota | lw1 | lw2
F8 = ml_dtypes.float8_e4m3

_cache = {}
_shard_cache = []


def _shard():
    if not _shard_cache:
        import jax
        from jax.sharding import Mesh, PartitionSpec, NamedSharding
        devices = jax.devices()[:NCORES]
        mesh = Mesh(np.asarray(devices), ("core",))
        _shard_cache.append(NamedSharding(mesh, PartitionSpec("core")))
    return _shard_cache[0]


def _build_program(CC):
    """Build + compile the SPMD bass program. CC: [2][NW] chunks per
    (source-half, dst-window); identical across cores."""
    from concourse import bacc, tile, mybir, library_config

    FOUT = [128, 128, 64, 32]
    FIN = [128, 128, 128, 64]
    f32, f16 = mybir.dt.float32, mybir.dt.float16
    i16, u8, f8 = mybir.dt.int16, mybir.dt.uint8, mybir.dt.float8e4

    n_chunks = int(CC.sum())
    NIDX = n_chunks * 128

    nc = bacc.Bacc("TRN2", target_bir_lowering=False, debug=False,
                   num_devices=NCORES)

    # --- dram parameters ---
    x_d = nc.dram_tensor("x", [SHP, 128], f8, kind="ExternalInput")
    idx_d = nc.dram_tensor("idx", [16, NIDX // 16], i16, kind="ExternalInput")
    d8_d = nc.dram_tensor("d8", [128, n_chunks], u8, kind="ExternalInput")
    nrmh_d = nc.dram_tensor("nrm", [128, n_chunks], f16, kind="ExternalInput")
    sc_d = nc.dram_tensor("sc", [128, 14], f32, kind="ExternalInput")
    wpk_d = nc.dram_tensor("wpk", [128, WPK], f16, kind="ExternalInput")
    # u8-quantized logits; cols SHP..SHP+8 hold per-feature min/max f32 bytes
    out_d = nc.dram_tensor("out", [40, SHP + 8], u8, kind="ExternalOutput")

    shard_d = [nc.dram_tensor(f"shard{l}", [SHP, 128], f16) for l in range(4)]
    table_d = [nc.dram_tensor(f"table{l}", [NP, 128], f16, addr_space="Shared")
               for l in range(4)]

    with tile.TileContext(nc) as tc:
        nc.gpsimd.load_library(library_config.mlp)
        with tc.tile_pool(name="pers", bufs=1) as pers, \
             tc.tile_pool(name="hpool", bufs=2) as hpool, \
             tc.tile_pool(name="zpool", bufs=2) as zpool, \
             tc.tile_pool(name="gpool", bufs=2) as gpool, \
             tc.tile_pool(name="mpool", bufs=6) as mpool, \
             tc.tile_pool(name="tpool", bufs=4) as tpool, \
             tc.tile_pool(name="wpool", bufs=3) as wpool, \
             tc.tile_pool(name="opool", bufs=2) as opool, \
             tc.tile_pool(name="ppa", bufs=3, space="PSUM") as ppa, \
             tc.tile_pool(name="ppw", bufs=2, space="PSUM") as ppw, \
             tc.tile_pool(name="pph1", bufs=1, space="PSUM") as pph1, \
             tc.tile_pool(name="pph2", bufs=1, space="PSUM") as pph2:

            # --- persistent loads ---
            idx_t = pers.tile([128, NIDX // 16], i16)
            for k in range(8):   # replicate [16, L] -> 8x in 128 partitions
                nc.sync.dma_start(out=idx_t[16 * k:16 * (k + 1), :],
                                  in_=idx_d[:])
            d8_t = pers.tile([128, n_chunks], u8)
            nc.sync.dma_start(out=d8_t[:], in_=d8_d[:])
            dst_t = pers.tile([128, n_chunks], f32)
            nc.scalar.copy(out=dst_t[:], in_=d8_t[:])
            nrmh_t = pers.tile([128, n_chunks], f16)
            nc.sync.dma_start(out=nrmh_t[:], in_=nrmh_d[:])
            nrm_t = pers.tile([128, n_chunks], f32)
            nc.scalar.copy(out=nrm_t[:], in_=nrmh_t[:])
            sc_t = pers.tile([128, 14], f32)
            nc.sync.dma_start(out=sc_t[:], in_=sc_d[:])
            wpk_t = pers.tile([128, WPK], f16)
            nc.sync.dma_start(out=wpk_t[:], in_=wpk_d[:])
            W_t = [wpk_t[0:128, l * 128:(l + 1) * 128] for l in range(4)]
            iota_t = wpk_t[0:128, 512:640]
            lw1_t = wpk_t[0:32, 640:704]
            lw2_t = wpk_t[0:64, 704:744]
            # identity (f8) for the PE x-transpose: id[p, j] = (j == p)
            id8_t = pers.tile([128, 128], f8)
            nc.vector.tensor_scalar(
                id8_t[:], iota_t, sc_t[:, 13:14], None,
                mybir.AluOpType.is_equal)

            # --- x load (f8, [node, feat]) + PE transpose + BN1 ---
            xs_t = pers.tile([128, NW, 128], f8)
            for t in range(NW):
                nc.sync.dma_start(out=xs_t[:, t, :],
                                  in_=x_d[t * 128:(t + 1) * 128, :])
            hprev = hpool.tile([128, SHP], f16, tag="h")
            for t in range(NW):
                pw = ppw.tile([128, 128], mybir.dt.float32, space="PSUM",
                              tag="pw")
                nc.tensor.matmul(out=pw[:], lhsT=xs_t[:, t, :], rhs=id8_t[:],
                                 start=True, stop=True)
                nc.scalar.activation(
                    out=hprev[:, t * 128:(t + 1) * 128], in_=pw[:],
                    func=mybir.ActivationFunctionType.Identity,
                    bias=sc_t[:, 1:2], scale=sc_t[:, 0:1])

            # window -> chunk index ranges per half
            half_windows = []   # [2][NW] -> (chunk_lo, chunk_hi) global ids
            cg = 0
            for h in range(2):
                rngs = []
                for w in range(NW):
                    rngs.append((cg, cg + int(CC[h][w])))
                    cg += int(CC[h][w])
                half_windows.append(rngs)

            for l in range(4):
                fin, fout = FIN[l], FOUT[l]
                # --- xW pass over own shard ---
                for t in range(NW):
                    pw = ppw.tile([128, 128], mybir.dt.float32, space="PSUM",
                                  tag="pw")
                    nc.tensor.matmul(
                        out=pw[0:128, 0:fout],
                        lhsT=hprev[0:fin, t * 128:(t + 1) * 128],
                        rhs=W_t[l][0:fin, 0:fout],
                        start=True, stop=True)
                    xw = wpool.tile([128, 128], f16, tag="xw")
                    nc.scalar.activation(
                        out=xw[0:128, 0:fout], in_=pw[0:128, 0:fout],
                        func=mybir.ActivationFunctionType.Copy)
                    nc.sync.dma_start(
                        out=shard_d[l][t * 128:(t + 1) * 128, 0:fout],
                        in_=xw[0:128, 0:fout])
                nc.gpsimd.collective_compute(
                    "AllGather", mybir.AluOpType.bypass,
                    replica_groups=[list(range(NCORES))],
                    ins=[shard_d[l][:]], outs=[table_d[l][:]])

                # --- aggregation pass ---
                z_t = zpool.tile([128, SHP], mybir.dt.float32, tag="z")
                hnext = hpool.tile([128, SHP], f16, tag="h")
                a_ap = sc_t[0:fout, 2 + 2 * l:3 + 2 * l]
                b_ap = sc_t[0:fout, 3 + 2 * l:4 + 2 * l]
                for h in range(2):
                    rngs = half_windows[h]
                    c_lo, c_hi = rngs[0][0], rngs[-1][1]
                    tab_ap = table_d[l][h * HALF:(h + 1) * HALF, :]
                    # super-gather batches
                    pa = None
                    w_idx = 0  # current window
                    for s0 in range(c_lo, c_hi, G):
                        s1 = min(s0 + G, c_hi)
                        ncur = s1 - s0
                        gt = gpool.tile([128, G, 128], f16, tag="g")
                        nc.gpsimd.dma_gather(
                            out_ap=gt[:, 0:ncur, :], in_ap=tab_ap,
                            idxs_ap=idx_t[:, s0 * 8:s1 * 8],
                            num_idxs=ncur * 128, num_idxs_reg=ncur * 128,
                            elem_size=128)
                        for j in range(ncur):
                            cgi = s0 + j
                            while cgi >= rngs[w_idx][1]:
                                w_idx += 1
                            w_lo, w_hi = rngs[w_idx]
                            mask = mpool.tile([128, 128], f16, tag="m")
                            nc.vector.tensor_scalar(
                                mask[:], iota_t,
                                dst_t[:, cgi:cgi + 1], nrm_t[:, cgi:cgi + 1],
                                mybir.AluOpType.is_equal,
                                mybir.AluOpType.mult)
                            if cgi == w_lo:
                                pa = ppa.tile([128, 128], mybir.dt.float32,
                                              space="PSUM", tag="pa")
                            nc.tensor.matmul(
                                out=pa[0:fout, :],
                                lhsT=gt[:, j, 0:fout], rhs=mask[:],
                                start=(cgi == w_lo), stop=(cgi == w_hi - 1))
                            if cgi == w_hi - 1:
                                wsl = slice(w_idx * 128, (w_idx + 1) * 128)
                                if h == 0:
                                    # stage a*psumA + b into f32 z
                                    nc.scalar.activation(
                                        out=z_t[0:fout, wsl],
                                        in_=pa[0:fout, :],
                                        func=mybir.ActivationFunctionType.Identity,
                                        bias=b_ap, scale=a_ap)
                                else:
                                    # combine + relu
                                    tmp = tpool.tile([128, 128], f16, tag="t")
                                    nc.vector.scalar_tensor_tensor(
                                        out=tmp[0:fout, :],
                                        in0=pa[0:fout, :], scalar=a_ap,
                                        in1=z_t[0:fout, wsl],
                                        op0=mybir.AluOpType.mult,
                                        op1=mybir.AluOpType.add)
                                    nc.scalar.activation(
                                        out=hnext[0:fout, wsl],
                                        in_=tmp[0:fout, :],
                                        func=mybir.ActivationFunctionType.Relu)
                hprev = hnext

            # --- head: relu already applied to hprev (=relu(bn5(agg4))) ---
            a6_ap = sc_t[0:64, 10:11]
            b6_ap = sc_t[0:64, 11:12]
            lb2_ap = sc_t[0:40, 12:13]
            hq = pers.tile([40, SHP], f16)
            for t0 in range(0, SHP, 512):
                L = min(512, SHP - t0)
                p1 = pph1.tile([64, 512], mybir.dt.float32, space="PSUM",
                               tag="p1")
                nc.tensor.matmul(out=p1[0:64, 0:L], lhsT=lw1_t,
                                 rhs=hprev[0:32, t0:t0 + L],
                                 start=True, stop=True)
                h5 = opool.tile([64, 512], f16, tag="h5")
                nc.scalar.activation(out=h5[0:64, 0:L], in_=p1[0:64, 0:L],
                                     func=mybir.ActivationFunctionType.Relu,
                                     bias=b6_ap, scale=a6_ap)
                p2 = pph2.tile([40, 512], mybir.dt.float32, space="PSUM",
                               tag="p2")
                nc.tensor.matmul(out=p2[0:40, 0:L], lhsT=lw2_t,
                                 rhs=h5[0:64, 0:L], start=True, stop=True)
                nc.scalar.activation(out=hq[0:40, t0:t0 + L],
                                     in_=p2[0:40, 0:L],
                                     func=mybir.ActivationFunctionType.Identity,
                                     bias=lb2_ap)

            # --- quantize logits to u8 with per-feature min/max ---
            mx_t = pers.tile([40, 1], f32)
            mn_t = pers.tile([40, 1], f32)
            nc.vector.tensor_reduce(out=mx_t[:], in_=hq[0:40, 0:SH],
                                    axis=mybir.AxisListType.X,
                                    op=mybir.AluOpType.max)
            nc.vector.tensor_reduce(out=mn_t[:], in_=hq[0:40, 0:SH],
                                    axis=mybir.AxisListType.X,
                                    op=mybir.AluOpType.min)
            d_t = pers.tile([40, 1], f32)
            nc.vector.tensor_tensor(d_t[:], mx_t[:], mn_t[:],
                                    mybir.AluOpType.subtract)
            r_t = pers.tile([40, 1], f32)
            nc.vector.reciprocal(out=r_t[:], in_=d_t[:])
            s_t = pers.tile([40, 1], f32)   # 255/(max-min)
            nc.vector.tensor_scalar_mul(s_t[:], r_t[:], 255.0)
            nb_t = pers.tile([40, 1], f32)  # -min*255/(max-min)
            nc.vector.scalar_tensor_tensor(
                out=nb_t[:], in0=mn_t[:], scalar=-1.0, in1=s_t[:],
                op0=mybir.AluOpType.mult, op1=mybir.AluOpType.mult)
            for t0 in range(0, SHP, 1568):
                L = min(1568, SHP - t0)
                q = opool.tile([40, 1568], u8, tag="q")
                nc.scalar.activation(out=q[0:40, 0:L],
                                     in_=hq[0:40, t0:t0 + L],
                                     func=mybir.ActivationFunctionType.Identity,
                                     bias=nb_t[:], scale=s_t[:])
                nc.sync.dma_start(out=out_d[0:40, t0:t0 + L],
                                  in_=q[0:40, 0:L])
            nc.sync.dma_start(out=out_d[0:40, SHP:SHP + 4],
                              in_=mn_t[:].bitcast(u8))
            nc.sync.dma_start(out=out_d[0:40, SHP + 4:SHP + 8],
                              in_=mx_t[:].bitcast(u8))
    nc.compile()
    return nc


class _Runner:
    """Caches the jitted PJRT executable for one compiled bass program."""

    def __init__(self, nc):
        import jax
        from jax.sharding import PartitionSpec
        from jax.experimental.shard_map import shard_map
        from concourse import mybir, bass2jax
        from concourse.bass2jax import _bass_exec_p, partition_id_tensor

        bass2jax.install_neuronx_cc_hook()
        self.jax = jax
        self.nc = nc
        self.shard = _shard()

        partition_name = (nc.partition_id_tensor.name
                          if nc.partition_id_tensor else None)
        in_names, out_names, out_avals, out_shapes = [], [], [], []
        for alloc in nc.m.functions[0].allocations:
            if not isinstance(alloc, mybir.MemoryLocationSet):
                continue
            name = alloc.memorylocations[0].name
            if alloc.kind == "ExternalInput":
                if name != partition_name:
                    in_names.append(name)
            elif alloc.kind == "ExternalOutput":
                shape = tuple(alloc.tensor_shape)
                dtype = mybir.dt.np(alloc.dtype)
                out_names.append(name)
                out_avals.append(jax.core.ShapedArray(shape, dtype))
                out_shapes.append((shape, dtype))
        self.in_names = list(in_names)
        self.out_names = list(out_names)
        n_params = len(in_names)
        n_outs = len(out_names)
        all_in = in_names + out_names + (
            [partition_name] if partition_name else [])

        def _body(*args):
            operands = list(args)
            if partition_name is not None:
                operands.append(partition_id_tensor())
            outs = _bass_exec_p.bind(
                *operands, out_avals=tuple(out_avals),
                in_names=tuple(all_in), out_names=tuple(out_names),
                lowering_input_output_aliases=(),
                sim_require_finite=True, sim_require_nnan=True, nc=nc)
            return tuple(outs)

        mesh = self.shard.mesh
        in_specs = (PartitionSpec("core"),) * (n_params + n_outs)
        out_specs = (PartitionSpec("core"),) * n_outs
        # The kernel fully writes its outputs, so the "pre-zeroed output"
        # operands are never read on device: skip donation and reuse one
        # persistent dummy buffer per output across calls.
        self.fn = jax.jit(
            shard_map(_body, mesh=mesh, in_specs=in_specs,
                      out_specs=out_specs, check_rep=False),
            keep_unused=True)
        self.z = tuple(
            jax.device_put(np.zeros((NCORES * s[0], *s[1:]), dt), self.shard)
            for s, dt in out_shapes)

    def __call__(self, arrays):
        args = [arrays[nm] for nm in self.in_names]
        outs = self.fn(*args, *self.z)
        return dict(zip(self.out_names, outs))

    def put(self, a):
        return self.jax.device_put(a, self.shard)


def _nb():
    """Lazily compile the numba helpers (cached on disk after first use)."""
    if "pass1" in _cache:
        return _cache["pass1"], _cache["pass2"]
    import numba

    @numba.njit(cache=True, fastmath=True)
    def pass1(src_e, dst_e, ew):
        deg = np.zeros(N, np.float64)
        gsz = np.zeros(NG, np.int64)
        for e in range(E + N):
            if e < E:
                s = src_e[e]
                d = dst_e[e]
                deg[d] += ew[e]
            else:
                s = e - E
                d = e - E
                deg[d] += 1.0
            half = 1 if s >= N // 2 else 0
            g = ((d // SH) * 2 + half) * NW + ((d % SH) >> 7)
            gsz[g] += 1
        return deg, gsz

    @numba.njit(cache=True, fastmath=True)
    def pass2(src_e, dst_e, ew, dinv, wp, IDX, D8, NRM32):
        for e in range(E + N):
            if e < E:
                s = src_e[e]
                d = dst_e[e]
                w = ew[e]
            else:
                s = e - E
                d = e - E
                w = np.float32(1.0)
            half = 1 if s >= N // 2 else 0
            cs = s // SH
            cd = d // SH
            dl = d - cd * SH
            g = (cd * 2 + half) * NW + (dl >> 7)
            p = wp[g]
            wp[g] = p + 1
            IDX[p] = np.int16((cs - (NCORES // 2) * half) * SHP +
                              (s - cs * SH))
            D8[p] = np.uint8(dl & 127)
            NRM32[p] = dinv[s] * w * dinv[d]

    _cache["pass1"] = pass1
    _cache["pass2"] = pass2
    return pass1, pass2


def _prep_edges(inputs):
    """Edge partitioning, normalization, packing (fused numba passes)."""
    pass1, pass2 = _nb()
    ei = np.asarray(inputs["edge_index"])
    ew = np.asarray(inputs["edge_weight"], np.float32)
    src_e = np.ascontiguousarray(ei[0])
    dst_e = np.ascontiguousarray(ei[1])

    deg, gsz = pass1(src_e, dst_e, ew)
    dinv = (1.0 / np.sqrt(np.maximum(deg, 1e-12))).astype(np.float32)

    CC = np.maximum(
        (gsz.reshape(NCORES, 2, NW).max(axis=0) + 127) // 128, 1)  # [2, NW]
    n_chunks = int(CC.sum())
    NIDX = n_chunks * 128

    # write cursor per group: core base + padded (half, window) offset
    pad = (CC.astype(np.int64) * 128).ravel()                # [2*NW]
    off = np.zeros(2 * NW, np.int64)
    off[1:] = np.cumsum(pad)[:-1]
    wp = (np.repeat(np.arange(NCORES, dtype=np.int64), 2 * NW) * NIDX +
          np.tile(off, NCORES))

    IDX = np.zeros(NCORES * NIDX, np.int16)
    D8 = np.zeros(NCORES * NIDX, np.uint8)
    NRM32 = np.zeros(NCORES * NIDX, np.float32)
    pass2(src_e, dst_e, ew, dinv, wp, IDX, D8, NRM32)
    NRM = NRM32.astype(np.float16)
    IDX = IDX.reshape(NCORES, NIDX)
    D8 = D8.reshape(NCORES, NIDX)
    NRM = NRM.reshape(NCORES, NIDX)

    idx_g = np.empty((NCORES * 16, NIDX // 16), np.int16)
    d8_g = np.empty((NCORES * 128, n_chunks), np.uint8)
    nrm_g = np.empty((NCORES * 128, n_chunks), np.float16)
    for c in range(NCORES):
        idx_g[c * 16:(c + 1) * 16] = IDX[c].reshape(-1, 16).T
        d8_g[c * 128:(c + 1) * 128] = D8[c].reshape(-1, 128).T
        nrm_g[c * 128:(c + 1) * 128] = NRM[c].reshape(-1, 128).T
    return CC, {"idx": idx_g, "d8": d8_g, "nrm": nrm_g}


def _prep_weights(inputs):
    """Small replicated tensors: packed weights + folded BN scale/bias."""
    FIN = [128, 128, 128, 64]
    FOUT = [128, 128, 64, 32]
    wpk = np.zeros((128, WPK), np.float16)
    for l in range(4):
        Wl = np.asarray(inputs[f"W{l+1}"], np.float32)
        wpk[:FIN[l], l * 128:l * 128 + FOUT[l]] = Wl.astype(np.float16)
    wpk[:, 512:640] = np.arange(128, dtype=np.float16)[None, :]
    wpk[:32, 640:704] = np.asarray(inputs["lw1"], np.float32).astype(np.float16)
    wpk[:64, 704:744] = np.asarray(inputs["lw2"], np.float32).astype(np.float16)

    sc = np.zeros((128, 14), np.float32)
    g1 = np.asarray(inputs["g1"], np.float32)
    s1 = g1 / np.sqrt(np.asarray(inputs["var1"], np.float32) + BN_EPS)
    sc[:, 0] = s1
    sc[:, 1] = np.asarray(inputs["beta1"], np.float32) - \
        np.asarray(inputs["mu1"], np.float32) * s1
    for l in range(4):
        bn = l + 2
        gl = np.asarray(inputs[f"g{bn}"], np.float32)
        a = gl / np.sqrt(np.asarray(inputs[f"var{bn}"], np.float32) + BN_EPS)
        b = (np.asarray(inputs[f"c{l+1}b"], np.float32) -
             np.asarray(inputs[f"mu{bn}"], np.float32)) * a + \
            np.asarray(inputs[f"beta{bn}"], np.float32)
        sc[:FOUT[l], 2 + 2 * l] = a
        sc[:FOUT[l], 3 + 2 * l] = b
    a6 = np.asarray(inputs["g6"], np.float32) / \
        np.sqrt(np.asarray(inputs["var6"], np.float32) + BN_EPS)
    b6 = (np.asarray(inputs["lb1"], np.float32) -
          np.asarray(inputs["mu6"], np.float32)) * a6 + \
        np.asarray(inputs["beta6"], np.float32)
    sc[:64, 10] = a6
    sc[:64, 11] = b6
    sc[:40, 12] = np.asarray(inputs["lb2"], np.float32)
    sc[:, 13] = np.arange(128, dtype=np.float32)

    scg = np.empty((NCORES * 128, 14), np.float32)
    wpkg = np.empty((NCORES * 128, WPK), np.float16)
    for c in range(NCORES):
        scg[c * 128:(c + 1) * 128] = sc
        wpkg[c * 128:(c + 1) * 128] = wpk
    return {"sc": scg, "wpk": wpkg}


def kernel(**inputs):
    import jax

    # cast + pad x to f8 and dispatch its upload first: the transfer
    # streams in the background while the edge preprocessing runs
    x = np.asarray(inputs["x"], np.float32)
    xp = np.zeros((NCORES * SHP, 128), F8)
    for c in range(NCORES):
        xp[c * SHP:c * SHP + SH] = x[c * SH:(c + 1) * SH]
    shard = _shard()
    arrays = {"x": jax.device_put(xp, shard)}
    for k, v in _prep_weights(inputs).items():
        arrays[k] = jax.device_put(v, shard)

    CC, edict = _prep_edges(inputs)
    key = CC.tobytes()
    if key not in _cache:
        nc = _build_program(CC)
        _cache[key] = _Runner(nc)
    run = _cache[key]
    for k, v in edict.items():
        arrays[k] = run.put(v)
    res = run(arrays)

    out_h = np.asarray(res["out"])     # [NCORES*40, SHP+8] u8
    out = np.empty((N, C), np.float32)
    for c in range(NCORES):
        blk = out_h[c * 40:(c + 1) * 40]
        mn = blk[:, SHP:SHP + 4].copy().view(np.float32)[:, 0]
        mx = blk[:, SHP + 4:SHP + 8].copy().view(np.float32)[:, 0]
        s = (mx - mn) * np.float32(1.0 / 255.0)
        out[c * SH:(c + 1) * SH] = \
            (blk[:, :SH] * s[:, None] + mn[:, None]).T
    return out


# revision 21
# speedup vs baseline: 1.1372x; 1.0605x over previous
"""4-layer GCN (N=50000, E=1.6M, F=128) on 8 Trainium2 NeuronCores.

Strategy:
  - Destination-node sharding: core c owns nodes [c*6250, (c+1)*6250).
  - Per layer: each core computes xW for its node shard (TensorE), shards are
    AllGathered into a full HBM feature table [50176, 128] fp16.
  - Message passing: edges (sorted by dst window) are gathered from the table
    via GPSIMD dma_gather (one 256B descriptor per edge); the weighted
    segment-sum over destinations is computed as mask matmuls on TensorE:
        mask[e, d] = norm_e * (dst_e == d)        (one DVE tensor_scalar op)
        psum[f, d] += gathered[e, f].T @ mask[e, d]   (fp32 PSUM accumulation)
  - Edges are split into two halves by source table row (int16 gather index
    limit); half A accumulates into PSUM and is staged to SBUF f32 (with the
    fused BN scale/bias), half B accumulates in PSUM and is combined with the
    staged value on DVE, then ReLU'd on ScalarE.
  - BatchNorms (eval mode) are folded into per-feature scale/bias applied on
    the PSUM->SBUF path. The final MLP head runs on-chip as well.

Host/transfer path (the wall-clock bottleneck on axon-tunneled cores):
  - The jitted PJRT executable is cached across kernel() calls (the stock
    run_bass_kernel_spmd re-traces and re-compiles the XLA wrapper per call).
  - Uploads are minimized: x ships as f8e4m3 (transposed on device via
    identity matmuls), gather indices ship unreplicated and are replicated
    16->128 partitions on device, dst-window as u8, edge norms as f16.
  - The kernel fully writes its output, so the pre-zeroed output operands
    the PJRT wrapper wants are never read: one persistent device-resident
    dummy is reused (no donation), saving a ~70ms dispatch round trip/call.
  - Output returns u8-quantized with per-feature min/max (computed on
    device) packed into the last 8 columns; the host dequantizes.
"""

import numpy as np
import ml_dtypes

N, E, F, C = 50000, 1600000, 128, 40
NCORES = 8
SH = N // NCORES            # 6250 nodes per core
NW = (SH + 127) // 128      # 49 dst windows per core
SHP = NW * 128              # 6272 padded shard rows
NP = NCORES * SHP           # 50176 padded table rows
HALF = NP // 2              # 25088 (int16-safe gather index range)
NG = NCORES * 2 * NW        # scatter groups
BN_EPS = 1e-5
G = 8                       # chunks (of 128 edges) per dma_gather call
                            # (SWDGE ring caps dma_gather at 1024 idxs/call)
WPK = 744                   # wpack cols: W1..W4 | iota | lw1 | lw2
F8 = ml_dtypes.float8_e4m3
XSPLIT = [1664, 1536, 1536, 1536]   # x upload quarters (rows, mult of 128)
NRM_K = 255.0 / 9.2103404           # log-u8 norm code: u8 = -ln(norm)*NRM_K

_cache = {}
_shard_cache = []


def _shard():
    if not _shard_cache:
        import jax
        from jax.sharding import Mesh, PartitionSpec, NamedSharding
        devices = jax.devices()[:NCORES]
        mesh = Mesh(np.asarray(devices), ("core",))
        _shard_cache.append(NamedSharding(mesh, PartitionSpec("core")))
    return _shard_cache[0]


def _build_program(CC):
    """Build + compile the SPMD bass program. CC: [2][NW] chunks per
    (source-half, dst-window); identical across cores."""
    from concourse import bacc, tile, mybir, library_config

    FOUT = [128, 128, 64, 32]
    FIN = [128, 128, 128, 64]
    f32, f16 = mybir.dt.float32, mybir.dt.float16
    i16, u8, f8 = mybir.dt.int16, mybir.dt.uint8, mybir.dt.float8e4

    n_chunks = int(CC.sum())
    NIDX = n_chunks * 128

    nc = bacc.Bacc("TRN2", target_bir_lowering=False, debug=False,
                   num_devices=NCORES)

    # --- dram parameters ---
    x_ds = [nc.dram_tensor(f"x{i}", [XSPLIT[i], 128], f8,
                           kind="ExternalInput") for i in range(4)]
    idx_d = nc.dram_tensor("idx", [16, NIDX // 16], i16, kind="ExternalInput")
    d8_d = nc.dram_tensor("d8", [128, n_chunks], u8, kind="ExternalInput")
    nrm8_d = nc.dram_tensor("nrm", [128, n_chunks], u8, kind="ExternalInput")
    sc_d = nc.dram_tensor("sc", [128, 14], f32, kind="ExternalInput")
    wpk_d = nc.dram_tensor("wpk", [128, WPK], f16, kind="ExternalInput")
    # u8-quantized logits; cols SHP..SHP+8 hold per-feature min/max f32 bytes
    out_d = nc.dram_tensor("out", [40, SHP + 8], u8, kind="ExternalOutput")

    shard_d = [nc.dram_tensor(f"shard{l}", [SHP, 128], f16) for l in range(4)]
    table_d = [nc.dram_tensor(f"table{l}", [NP, 128], f16, addr_space="Shared")
               for l in range(4)]

    with tile.TileContext(nc) as tc:
        nc.gpsimd.load_library(library_config.mlp)
        with tc.tile_pool(name="pers", bufs=1) as pers, \
             tc.tile_pool(name="hpool", bufs=2) as hpool, \
             tc.tile_pool(name="zpool", bufs=2) as zpool, \
             tc.tile_pool(name="gpool", bufs=2) as gpool, \
             tc.tile_pool(name="mpool", bufs=6) as mpool, \
             tc.tile_pool(name="tpool", bufs=4) as tpool, \
             tc.tile_pool(name="wpool", bufs=3) as wpool, \
             tc.tile_pool(name="opool", bufs=2) as opool, \
             tc.tile_pool(name="ppa", bufs=3, space="PSUM") as ppa, \
             tc.tile_pool(name="ppw", bufs=2, space="PSUM") as ppw, \
             tc.tile_pool(name="pph1", bufs=1, space="PSUM") as pph1, \
             tc.tile_pool(name="pph2", bufs=1, space="PSUM") as pph2:

            # --- persistent loads ---
            idx_t = pers.tile([128, NIDX // 16], i16)
            for k in range(8):   # replicate [16, L] -> 8x in 128 partitions
                nc.sync.dma_start(out=idx_t[16 * k:16 * (k + 1), :],
                                  in_=idx_d[:])
            d8_t = pers.tile([128, n_chunks], u8)
            nc.sync.dma_start(out=d8_t[:], in_=d8_d[:])
            dst_t = pers.tile([128, n_chunks], f32)
            nc.scalar.copy(out=dst_t[:], in_=d8_t[:])
            nrm8_t = pers.tile([128, n_chunks], u8)
            nc.sync.dma_start(out=nrm8_t[:], in_=nrm8_d[:])
            nrm_t = pers.tile([128, n_chunks], f32)
            nc.scalar.activation(out=nrm_t[:], in_=nrm8_t[:],
                                 func=mybir.ActivationFunctionType.Exp,
                                 scale=-1.0 / NRM_K)
            sc_t = pers.tile([128, 14], f32)
            nc.sync.dma_start(out=sc_t[:], in_=sc_d[:])
            wpk_t = pers.tile([128, WPK], f16)
            nc.sync.dma_start(out=wpk_t[:], in_=wpk_d[:])
            W_t = [wpk_t[0:128, l * 128:(l + 1) * 128] for l in range(4)]
            iota_t = wpk_t[0:128, 512:640]
            lw1_t = wpk_t[0:32, 640:704]
            lw2_t = wpk_t[0:64, 704:744]
            # identity (f8) for the PE x-transpose: id[p, j] = (j == p)
            id8_t = pers.tile([128, 128], f8)
            nc.vector.tensor_scalar(
                id8_t[:], iota_t, sc_t[:, 13:14], None,
                mybir.AluOpType.is_equal)

            # --- x load (f8, [node, feat]) + PE transpose + BN1 ---
            xs_t = pers.tile([128, NW, 128], f8)
            t = 0
            for i in range(4):
                for tt in range(XSPLIT[i] // 128):
                    nc.sync.dma_start(
                        out=xs_t[:, t, :],
                        in_=x_ds[i][tt * 128:(tt + 1) * 128, :])
                    t += 1
            hprev = hpool.tile([128, SHP], f16, tag="h")
            for t in range(NW):
                pw = ppw.tile([128, 128], mybir.dt.float32, space="PSUM",
                              tag="pw")
                nc.tensor.matmul(out=pw[:], lhsT=xs_t[:, t, :], rhs=id8_t[:],
                                 start=True, stop=True)
                nc.scalar.activation(
                    out=hprev[:, t * 128:(t + 1) * 128], in_=pw[:],
                    func=mybir.ActivationFunctionType.Identity,
                    bias=sc_t[:, 1:2], scale=sc_t[:, 0:1])

            # window -> chunk index ranges per half
            half_windows = []   # [2][NW] -> (chunk_lo, chunk_hi) global ids
            cg = 0
            for h in range(2):
                rngs = []
                for w in range(NW):
                    rngs.append((cg, cg + int(CC[h][w])))
                    cg += int(CC[h][w])
                half_windows.append(rngs)

            for l in range(4):
                fin, fout = FIN[l], FOUT[l]
                # --- xW pass over own shard ---
                for t in range(NW):
                    pw = ppw.tile([128, 128], mybir.dt.float32, space="PSUM",
                                  tag="pw")
                    nc.tensor.matmul(
                        out=pw[0:128, 0:fout],
                        lhsT=hprev[0:fin, t * 128:(t + 1) * 128],
                        rhs=W_t[l][0:fin, 0:fout],
                        start=True, stop=True)
                    xw = wpool.tile([128, 128], f16, tag="xw")
                    nc.scalar.activation(
                        out=xw[0:128, 0:fout], in_=pw[0:128, 0:fout],
                        func=mybir.ActivationFunctionType.Copy)
                    nc.sync.dma_start(
                        out=shard_d[l][t * 128:(t + 1) * 128, 0:fout],
                        in_=xw[0:128, 0:fout])
                nc.gpsimd.collective_compute(
                    "AllGather", mybir.AluOpType.bypass,
                    replica_groups=[list(range(NCORES))],
                    ins=[shard_d[l][:]], outs=[table_d[l][:]])

                # --- aggregation pass ---
                z_t = zpool.tile([128, SHP], mybir.dt.float32, tag="z")
                hnext = hpool.tile([128, SHP], f16, tag="h")
                a_ap = sc_t[0:fout, 2 + 2 * l:3 + 2 * l]
                b_ap = sc_t[0:fout, 3 + 2 * l:4 + 2 * l]
                for h in range(2):
                    rngs = half_windows[h]
                    c_lo, c_hi = rngs[0][0], rngs[-1][1]
                    tab_ap = table_d[l][h * HALF:(h + 1) * HALF, :]
                    # super-gather batches
                    pa = None
                    w_idx = 0  # current window
                    for s0 in range(c_lo, c_hi, G):
                        s1 = min(s0 + G, c_hi)
                        ncur = s1 - s0
                        gt = gpool.tile([128, G, 128], f16, tag="g")
                        nc.gpsimd.dma_gather(
                            out_ap=gt[:, 0:ncur, :], in_ap=tab_ap,
                            idxs_ap=idx_t[:, s0 * 8:s1 * 8],
                            num_idxs=ncur * 128, num_idxs_reg=ncur * 128,
                            elem_size=128)
                        for j in range(ncur):
                            cgi = s0 + j
                            while cgi >= rngs[w_idx][1]:
                                w_idx += 1
                            w_lo, w_hi = rngs[w_idx]
                            mask = mpool.tile([128, 128], f16, tag="m")
                            nc.vector.tensor_scalar(
                                mask[:], iota_t,
                                dst_t[:, cgi:cgi + 1], nrm_t[:, cgi:cgi + 1],
                                mybir.AluOpType.is_equal,
                                mybir.AluOpType.mult)
                            if cgi == w_lo:
                                pa = ppa.tile([128, 128], mybir.dt.float32,
                                              space="PSUM", tag="pa")
                            nc.tensor.matmul(
                                out=pa[0:fout, :],
                                lhsT=gt[:, j, 0:fout], rhs=mask[:],
                                start=(cgi == w_lo), stop=(cgi == w_hi - 1))
                            if cgi == w_hi - 1:
                                wsl = slice(w_idx * 128, (w_idx + 1) * 128)
                                if h == 0:
                                    # stage a*psumA + b into f32 z
                                    nc.scalar.activation(
                                        out=z_t[0:fout, wsl],
                                        in_=pa[0:fout, :],
                                        func=mybir.ActivationFunctionType.Identity,
                                        bias=b_ap, scale=a_ap)
                                else:
                                    # combine + relu
                                    tmp = tpool.tile([128, 128], f16, tag="t")
                                    nc.vector.scalar_tensor_tensor(
                                        out=tmp[0:fout, :],
                                        in0=pa[0:fout, :], scalar=a_ap,
                                        in1=z_t[0:fout, wsl],
                                        op0=mybir.AluOpType.mult,
                                        op1=mybir.AluOpType.add)
                                    nc.scalar.activation(
                                        out=hnext[0:fout, wsl],
                                        in_=tmp[0:fout, :],
                                        func=mybir.ActivationFunctionType.Relu)
                hprev = hnext

            # --- head: relu already applied to hprev (=relu(bn5(agg4))) ---
            a6_ap = sc_t[0:64, 10:11]
            b6_ap = sc_t[0:64, 11:12]
            lb2_ap = sc_t[0:40, 12:13]
            hq = pers.tile([40, SHP], f16)
            for t0 in range(0, SHP, 512):
                L = min(512, SHP - t0)
                p1 = pph1.tile([64, 512], mybir.dt.float32, space="PSUM",
                               tag="p1")
                nc.tensor.matmul(out=p1[0:64, 0:L], lhsT=lw1_t,
                                 rhs=hprev[0:32, t0:t0 + L],
                                 start=True, stop=True)
                h5 = opool.tile([64, 512], f16, tag="h5")
                nc.scalar.activation(out=h5[0:64, 0:L], in_=p1[0:64, 0:L],
                                     func=mybir.ActivationFunctionType.Relu,
                                     bias=b6_ap, scale=a6_ap)
                p2 = pph2.tile([40, 512], mybir.dt.float32, space="PSUM",
                               tag="p2")
                nc.tensor.matmul(out=p2[0:40, 0:L], lhsT=lw2_t,
                                 rhs=h5[0:64, 0:L], start=True, stop=True)
                nc.scalar.activation(out=hq[0:40, t0:t0 + L],
                                     in_=p2[0:40, 0:L],
                                     func=mybir.ActivationFunctionType.Identity,
                                     bias=lb2_ap)

            # --- quantize logits to u8 with per-feature min/max ---
            mx_t = pers.tile([40, 1], f32)
            mn_t = pers.tile([40, 1], f32)
            nc.vector.tensor_reduce(out=mx_t[:], in_=hq[0:40, 0:SH],
                                    axis=mybir.AxisListType.X,
                                    op=mybir.AluOpType.max)
            nc.vector.tensor_reduce(out=mn_t[:], in_=hq[0:40, 0:SH],
                                    axis=mybir.AxisListType.X,
                                    op=mybir.AluOpType.min)
            d_t = pers.tile([40, 1], f32)
            nc.vector.tensor_tensor(d_t[:], mx_t[:], mn_t[:],
                                    mybir.AluOpType.subtract)
            r_t = pers.tile([40, 1], f32)
            nc.vector.reciprocal(out=r_t[:], in_=d_t[:])
            s_t = pers.tile([40, 1], f32)   # 255/(max-min)
            nc.vector.tensor_scalar_mul(s_t[:], r_t[:], 255.0)
            nb_t = pers.tile([40, 1], f32)  # -min*255/(max-min)
            nc.vector.scalar_tensor_tensor(
                out=nb_t[:], in0=mn_t[:], scalar=-1.0, in1=s_t[:],
                op0=mybir.AluOpType.mult, op1=mybir.AluOpType.mult)
            for t0 in range(0, SHP, 1568):
                L = min(1568, SHP - t0)
                q = opool.tile([40, 1568], u8, tag="q")
                nc.scalar.activation(out=q[0:40, 0:L],
                                     in_=hq[0:40, t0:t0 + L],
                                     func=mybir.ActivationFunctionType.Identity,
                                     bias=nb_t[:], scale=s_t[:])
                nc.sync.dma_start(out=out_d[0:40, t0:t0 + L],
                                  in_=q[0:40, 0:L])
            nc.sync.dma_start(out=out_d[0:40, SHP:SHP + 4],
                              in_=mn_t[:].bitcast(u8))
            nc.sync.dma_start(out=out_d[0:40, SHP + 4:SHP + 8],
                              in_=mx_t[:].bitcast(u8))
    nc.compile()
    return nc


class _Runner:
    """Caches the jitted PJRT executable for one compiled bass program."""

    def __init__(self, nc):
        import jax
        from jax.sharding import PartitionSpec
        from jax.experimental.shard_map import shard_map
        from concourse import mybir, bass2jax
        from concourse.bass2jax import _bass_exec_p, partition_id_tensor

        bass2jax.install_neuronx_cc_hook()
        self.jax = jax
        self.nc = nc
        self.shard = _shard()

        partition_name = (nc.partition_id_tensor.name
                          if nc.partition_id_tensor else None)
        in_names, out_names, out_avals, out_shapes = [], [], [], []
        for alloc in nc.m.functions[0].allocations:
            if not isinstance(alloc, mybir.MemoryLocationSet):
                continue
            name = alloc.memorylocations[0].name
            if alloc.kind == "ExternalInput":
                if name != partition_name:
                    in_names.append(name)
            elif alloc.kind == "ExternalOutput":
                shape = tuple(alloc.tensor_shape)
                dtype = mybir.dt.np(alloc.dtype)
                out_names.append(name)
                out_avals.append(jax.core.ShapedArray(shape, dtype))
                out_shapes.append((shape, dtype))
        self.in_names = list(in_names)
        self.out_names = list(out_names)
        n_params = len(in_names)
        n_outs = len(out_names)
        all_in = in_names + out_names + (
            [partition_name] if partition_name else [])

        def _body(*args):
            operands = list(args)
            if partition_name is not None:
                operands.append(partition_id_tensor())
            outs = _bass_exec_p.bind(
                *operands, out_avals=tuple(out_avals),
                in_names=tuple(all_in), out_names=tuple(out_names),
                lowering_input_output_aliases=(),
                sim_require_finite=True, sim_require_nnan=True, nc=nc)
            return tuple(outs)

        mesh = self.shard.mesh
        in_specs = (PartitionSpec("core"),) * (n_params + n_outs)
        out_specs = (PartitionSpec("core"),) * n_outs
        # The kernel fully writes its outputs, so the "pre-zeroed output"
        # operands are never read on device: skip donation and reuse one
        # persistent dummy buffer per output across calls.
        self.fn = jax.jit(
            shard_map(_body, mesh=mesh, in_specs=in_specs,
                      out_specs=out_specs, check_rep=False),
            keep_unused=True)
        self.z = tuple(
            jax.device_put(np.zeros((NCORES * s[0], *s[1:]), dt), self.shard)
            for s, dt in out_shapes)

    def __call__(self, arrays):
        args = [arrays[nm] for nm in self.in_names]
        outs = self.fn(*args, *self.z)
        return dict(zip(self.out_names, outs))

    def put(self, a):
        return self.jax.device_put(a, self.shard)


def _nb():
    """Lazily compile the numba helpers (cached on disk after first use)."""
    if "pass1" in _cache:
        return _cache["pass1"], _cache["pass2"]
    import numba

    @numba.njit(cache=True, fastmath=True)
    def pass1(src_e, dst_e, ew):
        deg = np.zeros(N, np.float64)
        gsz = np.zeros(NG, np.int64)
        for e in range(E + N):
            if e < E:
                s = src_e[e]
                d = dst_e[e]
                deg[d] += ew[e]
            else:
                s = e - E
                d = e - E
                deg[d] += 1.0
            half = 1 if s >= N // 2 else 0
            g = ((d // SH) * 2 + half) * NW + ((d % SH) >> 7)
            gsz[g] += 1
        return deg, gsz

    @numba.njit(cache=True, fastmath=True)
    def pass2(src_e, dst_e, ew, dinv, wp, IDX, D8, NRM8):
        for e in range(E + N):
            if e < E:
                s = src_e[e]
                d = dst_e[e]
                w = ew[e]
            else:
                s = e - E
                d = e - E
                w = np.float32(1.0)
            half = 1 if s >= N // 2 else 0
            cs = s // SH
            cd = d // SH
            dl = d - cd * SH
            g = (cd * 2 + half) * NW + (dl >> 7)
            p = wp[g]
            wp[g] = p + 1
            IDX[p] = np.int16((cs - (NCORES // 2) * half) * SHP +
                              (s - cs * SH))
            D8[p] = np.uint8(dl & 127)
            nrm = dinv[s] * w * dinv[d]
            if nrm < 1e-4:
                nrm = 1e-4
            NRM8[p] = np.uint8(-np.log(nrm) * NRM_K + 0.5)

    _cache["pass1"] = pass1
    _cache["pass2"] = pass2
    return pass1, pass2


def _prep_edges(inputs):
    """Edge partitioning, normalization, packing (fused numba passes)."""
    pass1, pass2 = _nb()
    ei = np.asarray(inputs["edge_index"])
    ew = np.asarray(inputs["edge_weight"], np.float32)
    src_e = np.ascontiguousarray(ei[0])
    dst_e = np.ascontiguousarray(ei[1])

    deg, gsz = pass1(src_e, dst_e, ew)
    dinv = (1.0 / np.sqrt(np.maximum(deg, 1e-12))).astype(np.float32)

    CC = np.maximum(
        (gsz.reshape(NCORES, 2, NW).max(axis=0) + 127) // 128, 1)  # [2, NW]
    n_chunks = int(CC.sum())
    NIDX = n_chunks * 128

    # write cursor per group: core base + padded (half, window) offset
    pad = (CC.astype(np.int64) * 128).ravel()                # [2*NW]
    off = np.zeros(2 * NW, np.int64)
    off[1:] = np.cumsum(pad)[:-1]
    wp = (np.repeat(np.arange(NCORES, dtype=np.int64), 2 * NW) * NIDX +
          np.tile(off, NCORES))

    IDX = np.zeros(NCORES * NIDX, np.int16)
    # pad lanes get dst=255, which never matches the 0..127 iota -> mask 0
    D8 = np.full(NCORES * NIDX, 255, np.uint8)
    NRM = np.zeros(NCORES * NIDX, np.uint8)
    pass2(src_e, dst_e, ew, dinv, wp, IDX, D8, NRM)
    IDX = IDX.reshape(NCORES, NIDX)
    D8 = D8.reshape(NCORES, NIDX)
    NRM = NRM.reshape(NCORES, NIDX)

    idx_g = np.empty((NCORES * 16, NIDX // 16), np.int16)
    d8_g = np.empty((NCORES * 128, n_chunks), np.uint8)
    nrm_g = np.empty((NCORES * 128, n_chunks), np.uint8)
    for c in range(NCORES):
        idx_g[c * 16:(c + 1) * 16] = IDX[c].reshape(-1, 16).T
        d8_g[c * 128:(c + 1) * 128] = D8[c].reshape(-1, 128).T
        nrm_g[c * 128:(c + 1) * 128] = NRM[c].reshape(-1, 128).T
    return CC, {"idx": idx_g, "d8": d8_g, "nrm": nrm_g}


def _prep_weights(inputs):
    """Small replicated tensors: packed weights + folded BN scale/bias."""
    FIN = [128, 128, 128, 64]
    FOUT = [128, 128, 64, 32]
    wpk = np.zeros((128, WPK), np.float16)
    for l in range(4):
        Wl = np.asarray(inputs[f"W{l+1}"], np.float32)
        wpk[:FIN[l], l * 128:l * 128 + FOUT[l]] = Wl.astype(np.float16)
    wpk[:, 512:640] = np.arange(128, dtype=np.float16)[None, :]
    wpk[:32, 640:704] = np.asarray(inputs["lw1"], np.float32).astype(np.float16)
    wpk[:64, 704:744] = np.asarray(inputs["lw2"], np.float32).astype(np.float16)

    sc = np.zeros((128, 14), np.float32)
    g1 = np.asarray(inputs["g1"], np.float32)
    s1 = g1 / np.sqrt(np.asarray(inputs["var1"], np.float32) + BN_EPS)
    sc[:, 0] = s1
    sc[:, 1] = np.asarray(inputs["beta1"], np.float32) - \
        np.asarray(inputs["mu1"], np.float32) * s1
    for l in range(4):
        bn = l + 2
        gl = np.asarray(inputs[f"g{bn}"], np.float32)
        a = gl / np.sqrt(np.asarray(inputs[f"var{bn}"], np.float32) + BN_EPS)
        b = (np.asarray(inputs[f"c{l+1}b"], np.float32) -
             np.asarray(inputs[f"mu{bn}"], np.float32)) * a + \
            np.asarray(inputs[f"beta{bn}"], np.float32)
        sc[:FOUT[l], 2 + 2 * l] = a
        sc[:FOUT[l], 3 + 2 * l] = b
    a6 = np.asarray(inputs["g6"], np.float32) / \
        np.sqrt(np.asarray(inputs["var6"], np.float32) + BN_EPS)
    b6 = (np.asarray(inputs["lb1"], np.float32) -
          np.asarray(inputs["mu6"], np.float32)) * a6 + \
        np.asarray(inputs["beta6"], np.float32)
    sc[:64, 10] = a6
    sc[:64, 11] = b6
    sc[:40, 12] = np.asarray(inputs["lb2"], np.float32)
    sc[:, 13] = np.arange(128, dtype=np.float32)

    scg = np.empty((NCORES * 128, 14), np.float32)
    wpkg = np.empty((NCORES * 128, WPK), np.float16)
    for c in range(NCORES):
        scg[c * 128:(c + 1) * 128] = sc
        wpkg[c * 128:(c + 1) * 128] = wpk
    return {"sc": scg, "wpk": wpkg}


def kernel(**inputs):
    import jax

    # cast + pad x to f8 in quarters, dispatching each upload as soon as
    # it is cast so the wire starts early and streams during the edge prep
    x = np.asarray(inputs["x"], np.float32)
    shard = _shard()
    arrays = {}
    r0 = 0
    for i, nr in enumerate(XSPLIT):
        xq = np.zeros((NCORES * nr, 128), F8)
        hi = min(r0 + nr, SH)
        for c in range(NCORES):
            xq[c * nr:c * nr + hi - r0] = x[c * SH + r0:c * SH + hi]
        arrays[f"x{i}"] = jax.device_put(xq, shard)
        r0 += nr
    for k, v in _prep_weights(inputs).items():
        arrays[k] = jax.device_put(v, shard)

    CC, edict = _prep_edges(inputs)
    key = CC.tobytes()
    if key not in _cache:
        nc = _build_program(CC)
        _cache[key] = _Runner(nc)
    run = _cache[key]
    for k, v in edict.items():
        arrays[k] = run.put(v)
    res = run(arrays)

    out_h = np.asarray(res["out"])     # [NCORES*40, SHP+8] u8
    out = np.empty((N, C), np.float32)
    for c in range(NCORES):
        blk = out_h[c * 40:(c + 1) * 40]
        mn = blk[:, SHP:SHP + 4].copy().view(np.float32)[:, 0]
        mx = blk[:, SHP + 4:SHP + 8].copy().view(np.float32)[:, 0]
        s = (mx - mn) * np.float32(1.0 / 255.0)
        out[c * SH:(c + 1) * SH] = \
            (blk[:, :SH] * s[:, None] + mn[:, None]).T
    return out


# revision 24
# speedup vs baseline: 1.3178x; 1.1588x over previous
"""4-layer GCN (N=50000, E=1.6M, F=128) on 8 Trainium2 NeuronCores.

Strategy:
  - Destination-node sharding: core c owns nodes [c*6250, (c+1)*6250).
  - Per layer: each core computes xW for its node shard (TensorE), shards are
    AllGathered into a full HBM feature table [50176, 128] fp16.
  - Message passing: edges (sorted by dst window) are gathered from the table
    via GPSIMD dma_gather (one 256B descriptor per edge); the weighted
    segment-sum over destinations is computed as mask matmuls on TensorE:
        mask[e, d] = norm_e * (dst_e == d)        (one DVE tensor_scalar op)
        psum[f, d] += gathered[e, f].T @ mask[e, d]   (fp32 PSUM accumulation)
  - Edges are split into two halves by source table row (int16 gather index
    limit); half A accumulates into PSUM and is staged to SBUF f32 (with the
    fused BN scale/bias), half B accumulates in PSUM and is combined with the
    staged value on DVE, then ReLU'd on ScalarE.
  - BatchNorms (eval mode) are folded into per-feature scale/bias applied on
    the PSUM->SBUF path. The final MLP head runs on-chip as well.

Host/transfer path (the wall-clock bottleneck on axon-tunneled cores):
  - The jitted PJRT executable is cached across kernel() calls (the stock
    run_bass_kernel_spmd re-traces and re-compiles the XLA wrapper per call).
  - Uploads are minimized: x ships as f8e4m3 (transposed on device via
    identity matmuls), gather indices ship unreplicated and are replicated
    16->128 partitions on device, dst-window as u8, edge norms as f16.
  - The kernel fully writes its output, so the pre-zeroed output operands
    the PJRT wrapper wants are never read: one persistent device-resident
    dummy is reused (no donation), saving a ~70ms dispatch round trip/call.
  - Output returns u8-quantized with per-feature min/max (computed on
    device) packed into the last 8 columns; the host dequantizes.
"""

import numpy as np
import ml_dtypes

N, E, F, C = 50000, 1600000, 128, 40
NCORES = 8
SH = N // NCORES            # 6250 nodes per core
NW = (SH + 127) // 128      # 49 dst windows per core
SHP = NW * 128              # 6272 padded shard rows
NP = NCORES * SHP           # 50176 padded table rows
HALF = NP // 2              # 25088 (int16-safe gather index range)
NG = NCORES * 2 * NW        # scatter groups
BN_EPS = 1e-5
G = 8                       # chunks (of 128 edges) per dma_gather call
                            # (SWDGE ring caps dma_gather at 1024 idxs/call)
WPK = 744                   # wpack cols: W1..W4 | iota | lw1 | lw2
F8 = ml_dtypes.float8_e4m3
XSPLIT = [1664, 1536, 1536, 1536]   # x upload quarters (rows, mult of 128)
NRM_K = 255.0 / 9.2103404           # log-u8 norm code: u8 = -ln(norm)*NRM_K

_cache = {}
_shard_cache = []


def _shard():
    if not _shard_cache:
        import jax
        from jax.sharding import Mesh, PartitionSpec, NamedSharding
        devices = jax.devices()[:NCORES]
        mesh = Mesh(np.asarray(devices), ("core",))
        _shard_cache.append(NamedSharding(mesh, PartitionSpec("core")))
    return _shard_cache[0]


def _build_program(CC):
    """Build + compile the SPMD bass program. CC: [2][NW] chunks per
    (source-half, dst-window); identical across cores."""
    from concourse import bacc, tile, mybir, library_config

    FOUT = [128, 128, 64, 32]
    FIN = [128, 128, 128, 64]
    f32, f16 = mybir.dt.float32, mybir.dt.float16
    i16, u8, f8 = mybir.dt.int16, mybir.dt.uint8, mybir.dt.float8e4

    n_chunks = int(CC.sum())
    NIDX = n_chunks * 128

    nc = bacc.Bacc("TRN2", target_bir_lowering=False, debug=False,
                   num_devices=NCORES)

    # --- dram parameters ---
    x_ds = [nc.dram_tensor(f"x{i}", [XSPLIT[i], 128], f8,
                           kind="ExternalInput") for i in range(4)]
    idx_d = nc.dram_tensor("idx", [16, NIDX // 16], i16, kind="ExternalInput")
    d8_d = nc.dram_tensor("d8", [128, n_chunks], u8, kind="ExternalInput")
    nrm8_d = nc.dram_tensor("nrm", [128, n_chunks], u8, kind="ExternalInput")
    sc_d = nc.dram_tensor("sc", [128, 14], f32, kind="ExternalInput")
    wpk_d = nc.dram_tensor("wpk", [128, WPK], f16, kind="ExternalInput")
    # u8-quantized logits; cols SHP..SHP+8 hold per-feature min/max f32 bytes
    out_d = nc.dram_tensor("out", [40, SHP + 8], u8, kind="ExternalOutput")

    shard_d = [nc.dram_tensor(f"shard{l}", [SHP, 128], f16) for l in range(4)]
    table_d = [nc.dram_tensor(f"table{l}", [NP, 128], f16, addr_space="Shared")
               for l in range(4)]

    with tile.TileContext(nc) as tc:
        nc.gpsimd.load_library(library_config.mlp)
        with tc.tile_pool(name="pers", bufs=1) as pers, \
             tc.tile_pool(name="hpool", bufs=2) as hpool, \
             tc.tile_pool(name="zpool", bufs=2) as zpool, \
             tc.tile_pool(name="gpool", bufs=2) as gpool, \
             tc.tile_pool(name="mpool", bufs=6) as mpool, \
             tc.tile_pool(name="tpool", bufs=4) as tpool, \
             tc.tile_pool(name="wpool", bufs=3) as wpool, \
             tc.tile_pool(name="opool", bufs=2) as opool, \
             tc.tile_pool(name="ppa", bufs=3, space="PSUM") as ppa, \
             tc.tile_pool(name="ppw", bufs=2, space="PSUM") as ppw, \
             tc.tile_pool(name="pph1", bufs=1, space="PSUM") as pph1, \
             tc.tile_pool(name="pph2", bufs=1, space="PSUM") as pph2:

            # --- persistent loads ---
            idx_t = pers.tile([128, NIDX // 16], i16)
            for k in range(8):   # replicate [16, L] -> 8x in 128 partitions
                nc.sync.dma_start(out=idx_t[16 * k:16 * (k + 1), :],
                                  in_=idx_d[:])
            d8_t = pers.tile([128, n_chunks], u8)
            nc.sync.dma_start(out=d8_t[:], in_=d8_d[:])
            dst_t = pers.tile([128, n_chunks], f32)
            nc.scalar.copy(out=dst_t[:], in_=d8_t[:])
            nrm8_t = pers.tile([128, n_chunks], u8)
            nc.sync.dma_start(out=nrm8_t[:], in_=nrm8_d[:])
            nrm_t = pers.tile([128, n_chunks], f32)
            nc.scalar.activation(out=nrm_t[:], in_=nrm8_t[:],
                                 func=mybir.ActivationFunctionType.Exp,
                                 scale=-1.0 / NRM_K)
            sc_t = pers.tile([128, 14], f32)
            nc.sync.dma_start(out=sc_t[:], in_=sc_d[:])
            wpk_t = pers.tile([128, WPK], f16)
            nc.sync.dma_start(out=wpk_t[:], in_=wpk_d[:])
            W_t = [wpk_t[0:128, l * 128:(l + 1) * 128] for l in range(4)]
            iota_t = wpk_t[0:128, 512:640]
            lw1_t = wpk_t[0:32, 640:704]
            lw2_t = wpk_t[0:64, 704:744]
            # identity (f8) for the PE x-transpose: id[p, j] = (j == p)
            id8_t = pers.tile([128, 128], f8)
            nc.vector.tensor_scalar(
                id8_t[:], iota_t, sc_t[:, 13:14], None,
                mybir.AluOpType.is_equal)

            # --- x load (f8, [node, feat]) + PE transpose + BN1 ---
            xs_t = pers.tile([128, NW, 128], f8)
            t = 0
            for i in range(4):
                for tt in range(XSPLIT[i] // 128):
                    nc.sync.dma_start(
                        out=xs_t[:, t, :],
                        in_=x_ds[i][tt * 128:(tt + 1) * 128, :])
                    t += 1
            hprev = hpool.tile([128, SHP], f16, tag="h")
            for t in range(NW):
                pw = ppw.tile([128, 128], mybir.dt.float32, space="PSUM",
                              tag="pw")
                nc.tensor.matmul(out=pw[:], lhsT=xs_t[:, t, :], rhs=id8_t[:],
                                 start=True, stop=True)
                nc.scalar.activation(
                    out=hprev[:, t * 128:(t + 1) * 128], in_=pw[:],
                    func=mybir.ActivationFunctionType.Identity,
                    bias=sc_t[:, 1:2], scale=sc_t[:, 0:1])

            # window -> chunk index ranges per half
            half_windows = []   # [2][NW] -> (chunk_lo, chunk_hi) global ids
            cg = 0
            for h in range(2):
                rngs = []
                for w in range(NW):
                    rngs.append((cg, cg + int(CC[h][w])))
                    cg += int(CC[h][w])
                half_windows.append(rngs)

            for l in range(4):
                fin, fout = FIN[l], FOUT[l]
                # --- xW pass over own shard ---
                for t in range(NW):
                    pw = ppw.tile([128, 128], mybir.dt.float32, space="PSUM",
                                  tag="pw")
                    nc.tensor.matmul(
                        out=pw[0:128, 0:fout],
                        lhsT=hprev[0:fin, t * 128:(t + 1) * 128],
                        rhs=W_t[l][0:fin, 0:fout],
                        start=True, stop=True)
                    xw = wpool.tile([128, 128], f16, tag="xw")
                    nc.scalar.activation(
                        out=xw[0:128, 0:fout], in_=pw[0:128, 0:fout],
                        func=mybir.ActivationFunctionType.Copy)
                    nc.sync.dma_start(
                        out=shard_d[l][t * 128:(t + 1) * 128, 0:fout],
                        in_=xw[0:128, 0:fout])
                nc.gpsimd.collective_compute(
                    "AllGather", mybir.AluOpType.bypass,
                    replica_groups=[list(range(NCORES))],
                    ins=[shard_d[l][:]], outs=[table_d[l][:]])

                # --- aggregation pass ---
                z_t = zpool.tile([128, SHP], mybir.dt.float32, tag="z")
                hnext = hpool.tile([128, SHP], f16, tag="h")
                a_ap = sc_t[0:fout, 2 + 2 * l:3 + 2 * l]
                b_ap = sc_t[0:fout, 3 + 2 * l:4 + 2 * l]
                for h in range(2):
                    rngs = half_windows[h]
                    c_lo, c_hi = rngs[0][0], rngs[-1][1]
                    tab_ap = table_d[l][h * HALF:(h + 1) * HALF, :]
                    # super-gather batches
                    pa = None
                    w_idx = 0  # current window
                    for s0 in range(c_lo, c_hi, G):
                        s1 = min(s0 + G, c_hi)
                        ncur = s1 - s0
                        gt = gpool.tile([128, G, 128], f16, tag="g")
                        nc.gpsimd.dma_gather(
                            out_ap=gt[:, 0:ncur, :], in_ap=tab_ap,
                            idxs_ap=idx_t[:, s0 * 8:s1 * 8],
                            num_idxs=ncur * 128, num_idxs_reg=ncur * 128,
                            elem_size=128)
                        for j in range(ncur):
                            cgi = s0 + j
                            while cgi >= rngs[w_idx][1]:
                                w_idx += 1
                            w_lo, w_hi = rngs[w_idx]
                            mask = mpool.tile([128, 128], f16, tag="m")
                            nc.vector.tensor_scalar(
                                mask[:], iota_t,
                                dst_t[:, cgi:cgi + 1], nrm_t[:, cgi:cgi + 1],
                                mybir.AluOpType.is_equal,
                                mybir.AluOpType.mult)
                            if cgi == w_lo:
                                pa = ppa.tile([128, 128], mybir.dt.float32,
                                              space="PSUM", tag="pa")
                            nc.tensor.matmul(
                                out=pa[0:fout, :],
                                lhsT=gt[:, j, 0:fout], rhs=mask[:],
                                start=(cgi == w_lo), stop=(cgi == w_hi - 1))
                            if cgi == w_hi - 1:
                                wsl = slice(w_idx * 128, (w_idx + 1) * 128)
                                if h == 0:
                                    # stage a*psumA + b into f32 z
                                    nc.scalar.activation(
                                        out=z_t[0:fout, wsl],
                                        in_=pa[0:fout, :],
                                        func=mybir.ActivationFunctionType.Identity,
                                        bias=b_ap, scale=a_ap)
                                else:
                                    # combine + relu
                                    tmp = tpool.tile([128, 128], f16, tag="t")
                                    nc.vector.scalar_tensor_tensor(
                                        out=tmp[0:fout, :],
                                        in0=pa[0:fout, :], scalar=a_ap,
                                        in1=z_t[0:fout, wsl],
                                        op0=mybir.AluOpType.mult,
                                        op1=mybir.AluOpType.add)
                                    nc.scalar.activation(
                                        out=hnext[0:fout, wsl],
                                        in_=tmp[0:fout, :],
                                        func=mybir.ActivationFunctionType.Relu)
                hprev = hnext

            # --- head: relu already applied to hprev (=relu(bn5(agg4))) ---
            a6_ap = sc_t[0:64, 10:11]
            b6_ap = sc_t[0:64, 11:12]
            lb2_ap = sc_t[0:40, 12:13]
            hq = pers.tile([40, SHP], f16)
            for t0 in range(0, SHP, 512):
                L = min(512, SHP - t0)
                p1 = pph1.tile([64, 512], mybir.dt.float32, space="PSUM",
                               tag="p1")
                nc.tensor.matmul(out=p1[0:64, 0:L], lhsT=lw1_t,
                                 rhs=hprev[0:32, t0:t0 + L],
                                 start=True, stop=True)
                h5 = opool.tile([64, 512], f16, tag="h5")
                nc.scalar.activation(out=h5[0:64, 0:L], in_=p1[0:64, 0:L],
                                     func=mybir.ActivationFunctionType.Relu,
                                     bias=b6_ap, scale=a6_ap)
                p2 = pph2.tile([40, 512], mybir.dt.float32, space="PSUM",
                               tag="p2")
                nc.tensor.matmul(out=p2[0:40, 0:L], lhsT=lw2_t,
                                 rhs=h5[0:64, 0:L], start=True, stop=True)
                nc.scalar.activation(out=hq[0:40, t0:t0 + L],
                                     in_=p2[0:40, 0:L],
                                     func=mybir.ActivationFunctionType.Identity,
                                     bias=lb2_ap)

            # --- quantize logits to u8 with per-feature min/max ---
            mx_t = pers.tile([40, 1], f32)
            mn_t = pers.tile([40, 1], f32)
            nc.vector.tensor_reduce(out=mx_t[:], in_=hq[0:40, 0:SH],
                                    axis=mybir.AxisListType.X,
                                    op=mybir.AluOpType.max)
            nc.vector.tensor_reduce(out=mn_t[:], in_=hq[0:40, 0:SH],
                                    axis=mybir.AxisListType.X,
                                    op=mybir.AluOpType.min)
            d_t = pers.tile([40, 1], f32)
            nc.vector.tensor_tensor(d_t[:], mx_t[:], mn_t[:],
                                    mybir.AluOpType.subtract)
            r_t = pers.tile([40, 1], f32)
            nc.vector.reciprocal(out=r_t[:], in_=d_t[:])
            s_t = pers.tile([40, 1], f32)   # 255/(max-min)
            nc.vector.tensor_scalar_mul(s_t[:], r_t[:], 255.0)
            nb_t = pers.tile([40, 1], f32)  # -min*255/(max-min)
            nc.vector.scalar_tensor_tensor(
                out=nb_t[:], in0=mn_t[:], scalar=-1.0, in1=s_t[:],
                op0=mybir.AluOpType.mult, op1=mybir.AluOpType.mult)
            for t0 in range(0, SHP, 1568):
                L = min(1568, SHP - t0)
                q = opool.tile([40, 1568], u8, tag="q")
                nc.scalar.activation(out=q[0:40, 0:L],
                                     in_=hq[0:40, t0:t0 + L],
                                     func=mybir.ActivationFunctionType.Identity,
                                     bias=nb_t[:], scale=s_t[:])
                nc.sync.dma_start(out=out_d[0:40, t0:t0 + L],
                                  in_=q[0:40, 0:L])
            nc.sync.dma_start(out=out_d[0:40, SHP:SHP + 4],
                              in_=mn_t[:].bitcast(u8))
            nc.sync.dma_start(out=out_d[0:40, SHP + 4:SHP + 8],
                              in_=mx_t[:].bitcast(u8))
    nc.compile()
    return nc


class _Runner:
    """Caches the jitted PJRT executable for one compiled bass program."""

    def __init__(self, nc):
        import jax
        from jax.sharding import PartitionSpec
        from jax.experimental.shard_map import shard_map
        from concourse import mybir, bass2jax
        from concourse.bass2jax import _bass_exec_p, partition_id_tensor

        bass2jax.install_neuronx_cc_hook()
        self.jax = jax
        self.nc = nc
        self.shard = _shard()

        partition_name = (nc.partition_id_tensor.name
                          if nc.partition_id_tensor else None)
        in_names, out_names, out_avals, out_shapes = [], [], [], []
        for alloc in nc.m.functions[0].allocations:
            if not isinstance(alloc, mybir.MemoryLocationSet):
                continue
            name = alloc.memorylocations[0].name
            if alloc.kind == "ExternalInput":
                if name != partition_name:
                    in_names.append(name)
            elif alloc.kind == "ExternalOutput":
                shape = tuple(alloc.tensor_shape)
                dtype = mybir.dt.np(alloc.dtype)
                out_names.append(name)
                out_avals.append(jax.core.ShapedArray(shape, dtype))
                out_shapes.append((shape, dtype))
        self.in_names = list(in_names)
        self.out_names = list(out_names)
        n_params = len(in_names)
        n_outs = len(out_names)
        all_in = in_names + out_names + (
            [partition_name] if partition_name else [])

        def _body(*args):
            operands = list(args)
            if partition_name is not None:
                operands.append(partition_id_tensor())
            outs = _bass_exec_p.bind(
                *operands, out_avals=tuple(out_avals),
                in_names=tuple(all_in), out_names=tuple(out_names),
                lowering_input_output_aliases=(),
                sim_require_finite=True, sim_require_nnan=True, nc=nc)
            return tuple(outs)

        mesh = self.shard.mesh
        in_specs = (PartitionSpec("core"),) * (n_params + n_outs)
        out_specs = (PartitionSpec("core"),) * n_outs
        # The kernel fully writes its outputs, so the "pre-zeroed output"
        # operands are never read on device: skip donation and reuse one
        # persistent dummy buffer per output across calls.
        self.fn = jax.jit(
            shard_map(_body, mesh=mesh, in_specs=in_specs,
                      out_specs=out_specs, check_rep=False),
            keep_unused=True)
        self.z = tuple(
            jax.device_put(np.zeros((NCORES * s[0], *s[1:]), dt), self.shard)
            for s, dt in out_shapes)

    def __call__(self, arrays):
        args = [arrays[nm] for nm in self.in_names]
        outs = self.fn(*args, *self.z)
        return dict(zip(self.out_names, outs))

    def put(self, a):
        return self.jax.device_put(a, self.shard)


def _nb():
    """Lazily compile the numba helpers (cached on disk after first use)."""
    if "pass1" in _cache:
        return _cache["pass1"], _cache["pass2"]
    import numba

    @numba.njit(cache=True, fastmath=True)
    def pass1(src_e, dst_e, ew):
        # histogram over (group, dst-within-window): edges are emitted
        # dwin-sorted within each group, which turns the shipped d8 stream
        # into sorted runs the transfer tunnel's compressor crushes ~4x
        deg = np.zeros(N, np.float64)
        hist = np.zeros(NG * 128, np.int64)
        for e in range(E + N):
            if e < E:
                s = src_e[e]
                d = dst_e[e]
                deg[d] += ew[e]
            else:
                s = e - E
                d = e - E
                deg[d] += 1.0
            half = 1 if s >= N // 2 else 0
            dl = d % SH
            g = ((d // SH) * 2 + half) * NW + (dl >> 7)
            hist[g * 128 + (dl & 127)] += 1
        return deg, hist

    @numba.njit(cache=True, fastmath=True)
    def pass2(src_e, dst_e, ew, dinv, wp, IDX, D8, NRM8):
        for e in range(E + N):
            if e < E:
                s = src_e[e]
                d = dst_e[e]
                w = ew[e]
            else:
                s = e - E
                d = e - E
                w = np.float32(1.0)
            half = 1 if s >= N // 2 else 0
            cs = s // SH
            cd = d // SH
            dl = d - cd * SH
            g = (cd * 2 + half) * NW + (dl >> 7)
            b = g * 128 + (dl & 127)
            p = wp[b]
            wp[b] = p + 1
            IDX[p] = np.int16((cs - (NCORES // 2) * half) * SHP +
                              (s - cs * SH))
            D8[p] = np.uint8(dl & 127)
            nrm = dinv[s] * w * dinv[d]
            if nrm < 1e-4:
                nrm = 1e-4
            NRM8[p] = np.uint8(-np.log(nrm) * NRM_K + 0.5)

    _cache["pass1"] = pass1
    _cache["pass2"] = pass2
    return pass1, pass2


def _prep_edges(inputs):
    """Edge partitioning, normalization, packing (fused numba passes)."""
    pass1, pass2 = _nb()
    ei = np.asarray(inputs["edge_index"])
    ew = np.asarray(inputs["edge_weight"], np.float32)
    src_e = np.ascontiguousarray(ei[0])
    dst_e = np.ascontiguousarray(ei[1])

    deg, hist = pass1(src_e, dst_e, ew)
    dinv = (1.0 / np.sqrt(np.maximum(deg, 1e-12))).astype(np.float32)

    h2 = hist.reshape(NG, 128)
    gsz = h2.sum(axis=1)
    CC = np.maximum(
        (gsz.reshape(NCORES, 2, NW).max(axis=0) + 127) // 128, 1)  # [2, NW]
    n_chunks = int(CC.sum())
    NIDX = n_chunks * 128

    # write cursor per (group, dwin) bin: core base + padded (half, window)
    # offset + exclusive within-group prefix of smaller dwin values
    pad = (CC.astype(np.int64) * 128).ravel()                # [2*NW]
    off = np.zeros(2 * NW, np.int64)
    off[1:] = np.cumsum(pad)[:-1]
    wp0 = (np.repeat(np.arange(NCORES, dtype=np.int64), 2 * NW) * NIDX +
           np.tile(off, NCORES))
    excl = np.cumsum(h2, axis=1) - h2
    wp = (wp0[:, None] + excl).ravel()

    IDX = np.zeros(NCORES * NIDX, np.int16)
    # pad lanes get dst=255, which never matches the 0..127 iota -> mask 0
    D8 = np.full(NCORES * NIDX, 255, np.uint8)
    NRM = np.zeros(NCORES * NIDX, np.uint8)
    pass2(src_e, dst_e, ew, dinv, wp, IDX, D8, NRM)
    IDX = IDX.reshape(NCORES, NIDX)
    D8 = D8.reshape(NCORES, NIDX)
    NRM = NRM.reshape(NCORES, NIDX)

    idx_g = np.empty((NCORES * 16, NIDX // 16), np.int16)
    d8_g = np.empty((NCORES * 128, n_chunks), np.uint8)
    nrm_g = np.empty((NCORES * 128, n_chunks), np.uint8)
    for c in range(NCORES):
        idx_g[c * 16:(c + 1) * 16] = IDX[c].reshape(-1, 16).T
        d8_g[c * 128:(c + 1) * 128] = D8[c].reshape(-1, 128).T
        nrm_g[c * 128:(c + 1) * 128] = NRM[c].reshape(-1, 128).T
    return CC, {"idx": idx_g, "d8": d8_g, "nrm": nrm_g}


def _prep_weights(inputs):
    """Small replicated tensors: packed weights + folded BN scale/bias."""
    FIN = [128, 128, 128, 64]
    FOUT = [128, 128, 64, 32]
    wpk = np.zeros((128, WPK), np.float16)
    for l in range(4):
        Wl = np.asarray(inputs[f"W{l+1}"], np.float32)
        wpk[:FIN[l], l * 128:l * 128 + FOUT[l]] = Wl.astype(np.float16)
    wpk[:, 512:640] = np.arange(128, dtype=np.float16)[None, :]
    wpk[:32, 640:704] = np.asarray(inputs["lw1"], np.float32).astype(np.float16)
    wpk[:64, 704:744] = np.asarray(inputs["lw2"], np.float32).astype(np.float16)

    sc = np.zeros((128, 14), np.float32)
    g1 = np.asarray(inputs["g1"], np.float32)
    s1 = g1 / np.sqrt(np.asarray(inputs["var1"], np.float32) + BN_EPS)
    sc[:, 0] = s1
    sc[:, 1] = np.asarray(inputs["beta1"], np.float32) - \
        np.asarray(inputs["mu1"], np.float32) * s1
    for l in range(4):
        bn = l + 2
        gl = np.asarray(inputs[f"g{bn}"], np.float32)
        a = gl / np.sqrt(np.asarray(inputs[f"var{bn}"], np.float32) + BN_EPS)
        b = (np.asarray(inputs[f"c{l+1}b"], np.float32) -
             np.asarray(inputs[f"mu{bn}"], np.float32)) * a + \
            np.asarray(inputs[f"beta{bn}"], np.float32)
        sc[:FOUT[l], 2 + 2 * l] = a
        sc[:FOUT[l], 3 + 2 * l] = b
    a6 = np.asarray(inputs["g6"], np.float32) / \
        np.sqrt(np.asarray(inputs["var6"], np.float32) + BN_EPS)
    b6 = (np.asarray(inputs["lb1"], np.float32) -
          np.asarray(inputs["mu6"], np.float32)) * a6 + \
        np.asarray(inputs["beta6"], np.float32)
    sc[:64, 10] = a6
    sc[:64, 11] = b6
    sc[:40, 12] = np.asarray(inputs["lb2"], np.float32)
    sc[:, 13] = np.arange(128, dtype=np.float32)

    scg = np.empty((NCORES * 128, 14), np.float32)
    wpkg = np.empty((NCORES * 128, WPK), np.float16)
    for c in range(NCORES):
        scg[c * 128:(c + 1) * 128] = sc
        wpkg[c * 128:(c + 1) * 128] = wpk
    return {"sc": scg, "wpk": wpkg}


def kernel(**inputs):
    import jax

    # cast + pad x to f8 in quarters, dispatching each upload as soon as
    # it is cast so the wire starts early and streams during the edge prep
    x = np.asarray(inputs["x"], np.float32)
    shard = _shard()
    arrays = {}
    r0 = 0
    for i, nr in enumerate(XSPLIT):
        xq = np.zeros((NCORES * nr, 128), F8)
        hi = min(r0 + nr, SH)
        for c in range(NCORES):
            xq[c * nr:c * nr + hi - r0] = x[c * SH + r0:c * SH + hi]
        arrays[f"x{i}"] = jax.device_put(xq, shard)
        r0 += nr
    for k, v in _prep_weights(inputs).items():
        arrays[k] = jax.device_put(v, shard)

    CC, edict = _prep_edges(inputs)
    key = CC.tobytes()
    if key not in _cache:
        nc = _build_program(CC)
        _cache[key] = _Runner(nc)
    run = _cache[key]
    for k, v in edict.items():
        arrays[k] = run.put(v)
    res = run(arrays)

    out_h = np.asarray(res["out"])     # [NCORES*40, SHP+8] u8
    out = np.empty((N, C), np.float32)
    for c in range(NCORES):
        blk = out_h[c * 40:(c + 1) * 40]
        mn = blk[:, SHP:SHP + 4].copy().view(np.float32)[:, 0]
        mx = blk[:, SHP + 4:SHP + 8].copy().view(np.float32)[:, 0]
        s = (mx - mn) * np.float32(1.0 / 255.0)
        out[c * SH:(c + 1) * SH] = \
            (blk[:, :SH] * s[:, None] + mn[:, None]).T
    return out


# revision 25
# speedup vs baseline: 1.3609x; 1.0327x over previous
"""4-layer GCN (N=50000, E=1.6M, F=128) on 8 Trainium2 NeuronCores.

Strategy:
  - Destination-node sharding: core c owns nodes [c*6250, (c+1)*6250).
  - Per layer: each core computes xW for its node shard (TensorE), shards are
    AllGathered into a full HBM feature table [50176, 128] fp16.
  - Message passing: edges (sorted by dst window) are gathered from the table
    via GPSIMD dma_gather (one 256B descriptor per edge); the weighted
    segment-sum over destinations is computed as mask matmuls on TensorE:
        mask[e, d] = norm_e * (dst_e == d)        (one DVE tensor_scalar op)
        psum[f, d] += gathered[e, f].T @ mask[e, d]   (fp32 PSUM accumulation)
  - Edges are split into two halves by source table row (int16 gather index
    limit); half A accumulates into PSUM and is staged to SBUF f32 (with the
    fused BN scale/bias), half B accumulates in PSUM and is combined with the
    staged value on DVE, then ReLU'd on ScalarE.
  - BatchNorms (eval mode) are folded into per-feature scale/bias applied on
    the PSUM->SBUF path. The final MLP head runs on-chip as well.

Host/transfer path (the wall-clock bottleneck on axon-tunneled cores):
  - The jitted PJRT executable is cached across kernel() calls (the stock
    run_bass_kernel_spmd re-traces and re-compiles the XLA wrapper per call).
  - Uploads are minimized: x ships as f8e4m3 (transposed on device via
    identity matmuls), gather indices ship unreplicated and are replicated
    16->128 partitions on device, dst-window as u8, edge norms as f16.
  - The kernel fully writes its output, so the pre-zeroed output operands
    the PJRT wrapper wants are never read: one persistent device-resident
    dummy is reused (no donation), saving a ~70ms dispatch round trip/call.
  - Output returns u8-quantized with per-feature min/max (computed on
    device) packed into the last 8 columns; the host dequantizes.
"""

import numpy as np
import ml_dtypes

N, E, F, C = 50000, 1600000, 128, 40
NCORES = 8
SH = N // NCORES            # 6250 nodes per core
NW = (SH + 127) // 128      # 49 dst windows per core
SHP = NW * 128              # 6272 padded shard rows
NP = NCORES * SHP           # 50176 padded table rows
HALF = NP // 2              # 25088 (int16-safe gather index range)
NG = NCORES * 2 * NW        # scatter groups
BN_EPS = 1e-5
G = 8                       # chunks (of 128 edges) per dma_gather call
                            # (SWDGE ring caps dma_gather at 1024 idxs/call)
WPK = 744                   # wpack cols: W1..W4 | iota | lw1 | lw2
F8 = ml_dtypes.float8_e4m3
XSPLIT = [1664, 1536, 1536, 1536]   # x upload quarters (rows, mult of 128)
NRM_K = 255.0 / 9.2103404           # log-u8 norm code: u8 = -ln(norm)*NRM_K

_cache = {}
_shard_cache = []


def _shard():
    if not _shard_cache:
        import jax
        from jax.sharding import Mesh, PartitionSpec, NamedSharding
        devices = jax.devices()[:NCORES]
        mesh = Mesh(np.asarray(devices), ("core",))
        _shard_cache.append(NamedSharding(mesh, PartitionSpec("core")))
    return _shard_cache[0]


def _build_program(CC):
    """Build + compile the SPMD bass program. CC: [2][NW] chunks per
    (source-half, dst-window); identical across cores."""
    from concourse import bacc, tile, mybir, library_config

    FOUT = [128, 128, 64, 32]
    FIN = [128, 128, 128, 64]
    f32, f16 = mybir.dt.float32, mybir.dt.float16
    i16, u8, f8 = mybir.dt.int16, mybir.dt.uint8, mybir.dt.float8e4

    n_chunks = int(CC.sum())
    NIDX = n_chunks * 128

    nc = bacc.Bacc("TRN2", target_bir_lowering=False, debug=False,
                   num_devices=NCORES)

    # --- dram parameters ---
    x_ds = [nc.dram_tensor(f"x{i}", [XSPLIT[i], 128], f8,
                           kind="ExternalInput") for i in range(4)]
    idx_d = nc.dram_tensor("idx", [16, NIDX // 16], i16, kind="ExternalInput")
    d8_d = nc.dram_tensor("d8", [128, n_chunks], u8, kind="ExternalInput")
    nrm8_d = nc.dram_tensor("nrm", [128, n_chunks], u8, kind="ExternalInput")
    sc_d = nc.dram_tensor("sc", [128, 14], f32, kind="ExternalInput")
    wpk_d = nc.dram_tensor("wpk", [128, WPK], f16, kind="ExternalInput")
    # u8-quantized logits; cols SHP..SHP+8 hold per-feature min/max f32 bytes
    out_d = nc.dram_tensor("out", [40, SHP + 8], u8, kind="ExternalOutput")

    shard_d = [nc.dram_tensor(f"shard{l}", [SHP, 128], f16) for l in range(4)]
    table_d = [nc.dram_tensor(f"table{l}", [NP, 128], f16, addr_space="Shared")
               for l in range(4)]

    with tile.TileContext(nc) as tc:
        nc.gpsimd.load_library(library_config.mlp)
        with tc.tile_pool(name="pers", bufs=1) as pers, \
             tc.tile_pool(name="hpool", bufs=2) as hpool, \
             tc.tile_pool(name="zpool", bufs=2) as zpool, \
             tc.tile_pool(name="gpool", bufs=2) as gpool, \
             tc.tile_pool(name="mpool", bufs=6) as mpool, \
             tc.tile_pool(name="tpool", bufs=4) as tpool, \
             tc.tile_pool(name="wpool", bufs=3) as wpool, \
             tc.tile_pool(name="opool", bufs=2) as opool, \
             tc.tile_pool(name="ppa", bufs=3, space="PSUM") as ppa, \
             tc.tile_pool(name="ppw", bufs=2, space="PSUM") as ppw, \
             tc.tile_pool(name="pph1", bufs=1, space="PSUM") as pph1, \
             tc.tile_pool(name="pph2", bufs=1, space="PSUM") as pph2:

            # --- persistent loads ---
            idx_t = pers.tile([128, NIDX // 16], i16)
            for k in range(8):   # replicate [16, L] -> 8x in 128 partitions
                nc.sync.dma_start(out=idx_t[16 * k:16 * (k + 1), :],
                                  in_=idx_d[:])
            d8_t = pers.tile([128, n_chunks], u8)
            nc.sync.dma_start(out=d8_t[:], in_=d8_d[:])
            dst_t = pers.tile([128, n_chunks], f32)
            nc.scalar.copy(out=dst_t[:], in_=d8_t[:])
            nrm8_t = pers.tile([128, n_chunks], u8)
            nc.sync.dma_start(out=nrm8_t[:], in_=nrm8_d[:])
            nrm_t = pers.tile([128, n_chunks], f32)
            nc.scalar.activation(out=nrm_t[:], in_=nrm8_t[:],
                                 func=mybir.ActivationFunctionType.Exp,
                                 scale=-1.0 / NRM_K)
            sc_t = pers.tile([128, 14], f32)
            nc.sync.dma_start(out=sc_t[:], in_=sc_d[:])
            wpk_t = pers.tile([128, WPK], f16)
            nc.sync.dma_start(out=wpk_t[:], in_=wpk_d[:])
            W_t = [wpk_t[0:128, l * 128:(l + 1) * 128] for l in range(4)]
            iota_t = wpk_t[0:128, 512:640]
            lw1_t = wpk_t[0:32, 640:704]
            lw2_t = wpk_t[0:64, 704:744]
            # identity (f8) for the PE x-transpose: id[p, j] = (j == p)
            id8_t = pers.tile([128, 128], f8)
            nc.vector.tensor_scalar(
                id8_t[:], iota_t, sc_t[:, 13:14], None,
                mybir.AluOpType.is_equal)

            # --- x load (f8, [node, feat]) + PE transpose + BN1 ---
            xs_t = pers.tile([128, NW, 128], f8)
            t = 0
            for i in range(4):
                for tt in range(XSPLIT[i] // 128):
                    nc.sync.dma_start(
                        out=xs_t[:, t, :],
                        in_=x_ds[i][tt * 128:(tt + 1) * 128, :])
                    t += 1
            hprev = hpool.tile([128, SHP], f16, tag="h")
            for t in range(NW):
                pw = ppw.tile([128, 128], mybir.dt.float32, space="PSUM",
                              tag="pw")
                nc.tensor.matmul(out=pw[:], lhsT=xs_t[:, t, :], rhs=id8_t[:],
                                 start=True, stop=True)
                nc.scalar.activation(
                    out=hprev[:, t * 128:(t + 1) * 128], in_=pw[:],
                    func=mybir.ActivationFunctionType.Identity,
                    bias=sc_t[:, 1:2], scale=sc_t[:, 0:1])

            # window -> chunk index ranges per half
            half_windows = []   # [2][NW] -> (chunk_lo, chunk_hi) global ids
            cg = 0
            for h in range(2):
                rngs = []
                for w in range(NW):
                    rngs.append((cg, cg + int(CC[h][w])))
                    cg += int(CC[h][w])
                half_windows.append(rngs)

            for l in range(4):
                fin, fout = FIN[l], FOUT[l]
                # --- xW pass over own shard ---
                for t in range(NW):
                    pw = ppw.tile([128, 128], mybir.dt.float32, space="PSUM",
                                  tag="pw")
                    nc.tensor.matmul(
                        out=pw[0:128, 0:fout],
                        lhsT=hprev[0:fin, t * 128:(t + 1) * 128],
                        rhs=W_t[l][0:fin, 0:fout],
                        start=True, stop=True)
                    xw = wpool.tile([128, 128], f16, tag="xw")
                    nc.scalar.activation(
                        out=xw[0:128, 0:fout], in_=pw[0:128, 0:fout],
                        func=mybir.ActivationFunctionType.Copy)
                    nc.sync.dma_start(
                        out=shard_d[l][t * 128:(t + 1) * 128, 0:fout],
                        in_=xw[0:128, 0:fout])
                nc.gpsimd.collective_compute(
                    "AllGather", mybir.AluOpType.bypass,
                    replica_groups=[list(range(NCORES))],
                    ins=[shard_d[l][:]], outs=[table_d[l][:]])

                # --- aggregation pass ---
                z_t = zpool.tile([128, SHP], mybir.dt.float32, tag="z")
                hnext = hpool.tile([128, SHP], f16, tag="h")
                a_ap = sc_t[0:fout, 2 + 2 * l:3 + 2 * l]
                b_ap = sc_t[0:fout, 3 + 2 * l:4 + 2 * l]
                for h in range(2):
                    rngs = half_windows[h]
                    c_lo, c_hi = rngs[0][0], rngs[-1][1]
                    tab_ap = table_d[l][h * HALF:(h + 1) * HALF, :]
                    # super-gather batches
                    pa = None
                    w_idx = 0  # current window
                    for s0 in range(c_lo, c_hi, G):
                        s1 = min(s0 + G, c_hi)
                        ncur = s1 - s0
                        gt = gpool.tile([128, G, 128], f16, tag="g")
                        nc.gpsimd.dma_gather(
                            out_ap=gt[:, 0:ncur, :], in_ap=tab_ap,
                            idxs_ap=idx_t[:, s0 * 8:s1 * 8],
                            num_idxs=ncur * 128, num_idxs_reg=ncur * 128,
                            elem_size=128)
                        for j in range(ncur):
                            cgi = s0 + j
                            while cgi >= rngs[w_idx][1]:
                                w_idx += 1
                            w_lo, w_hi = rngs[w_idx]
                            mask = mpool.tile([128, 128], f16, tag="m")
                            nc.vector.tensor_scalar(
                                mask[:], iota_t,
                                dst_t[:, cgi:cgi + 1], nrm_t[:, cgi:cgi + 1],
                                mybir.AluOpType.is_equal,
                                mybir.AluOpType.mult)
                            if cgi == w_lo:
                                pa = ppa.tile([128, 128], mybir.dt.float32,
                                              space="PSUM", tag="pa")
                            nc.tensor.matmul(
                                out=pa[0:fout, :],
                                lhsT=gt[:, j, 0:fout], rhs=mask[:],
                                start=(cgi == w_lo), stop=(cgi == w_hi - 1))
                            if cgi == w_hi - 1:
                                wsl = slice(w_idx * 128, (w_idx + 1) * 128)
                                if h == 0:
                                    # stage a*psumA + b into f32 z
                                    nc.scalar.activation(
                                        out=z_t[0:fout, wsl],
                                        in_=pa[0:fout, :],
                                        func=mybir.ActivationFunctionType.Identity,
                                        bias=b_ap, scale=a_ap)
                                else:
                                    # combine + relu
                                    tmp = tpool.tile([128, 128], f16, tag="t")
                                    nc.vector.scalar_tensor_tensor(
                                        out=tmp[0:fout, :],
                                        in0=pa[0:fout, :], scalar=a_ap,
                                        in1=z_t[0:fout, wsl],
                                        op0=mybir.AluOpType.mult,
                                        op1=mybir.AluOpType.add)
                                    nc.scalar.activation(
                                        out=hnext[0:fout, wsl],
                                        in_=tmp[0:fout, :],
                                        func=mybir.ActivationFunctionType.Relu)
                hprev = hnext

            # --- head: relu already applied to hprev (=relu(bn5(agg4))) ---
            a6_ap = sc_t[0:64, 10:11]
            b6_ap = sc_t[0:64, 11:12]
            lb2_ap = sc_t[0:40, 12:13]
            hq = pers.tile([40, SHP], f16)
            for t0 in range(0, SHP, 512):
                L = min(512, SHP - t0)
                p1 = pph1.tile([64, 512], mybir.dt.float32, space="PSUM",
                               tag="p1")
                nc.tensor.matmul(out=p1[0:64, 0:L], lhsT=lw1_t,
                                 rhs=hprev[0:32, t0:t0 + L],
                                 start=True, stop=True)
                h5 = opool.tile([64, 512], f16, tag="h5")
                nc.scalar.activation(out=h5[0:64, 0:L], in_=p1[0:64, 0:L],
                                     func=mybir.ActivationFunctionType.Relu,
                                     bias=b6_ap, scale=a6_ap)
                p2 = pph2.tile([40, 512], mybir.dt.float32, space="PSUM",
                               tag="p2")
                nc.tensor.matmul(out=p2[0:40, 0:L], lhsT=lw2_t,
                                 rhs=h5[0:64, 0:L], start=True, stop=True)
                nc.scalar.activation(out=hq[0:40, t0:t0 + L],
                                     in_=p2[0:40, 0:L],
                                     func=mybir.ActivationFunctionType.Identity,
                                     bias=lb2_ap)

            # --- quantize logits to u8 with per-feature min/max ---
            mx_t = pers.tile([40, 1], f32)
            mn_t = pers.tile([40, 1], f32)
            nc.vector.tensor_reduce(out=mx_t[:], in_=hq[0:40, 0:SH],
                                    axis=mybir.AxisListType.X,
                                    op=mybir.AluOpType.max)
            nc.vector.tensor_reduce(out=mn_t[:], in_=hq[0:40, 0:SH],
                                    axis=mybir.AxisListType.X,
                                    op=mybir.AluOpType.min)
            d_t = pers.tile([40, 1], f32)
            nc.vector.tensor_tensor(d_t[:], mx_t[:], mn_t[:],
                                    mybir.AluOpType.subtract)
            r_t = pers.tile([40, 1], f32)
            nc.vector.reciprocal(out=r_t[:], in_=d_t[:])
            s_t = pers.tile([40, 1], f32)   # 255/(max-min)
            nc.vector.tensor_scalar_mul(s_t[:], r_t[:], 255.0)
            nb_t = pers.tile([40, 1], f32)  # -min*255/(max-min)
            nc.vector.scalar_tensor_tensor(
                out=nb_t[:], in0=mn_t[:], scalar=-1.0, in1=s_t[:],
                op0=mybir.AluOpType.mult, op1=mybir.AluOpType.mult)
            for t0 in range(0, SHP, 1568):
                L = min(1568, SHP - t0)
                q = opool.tile([40, 1568], u8, tag="q")
                nc.scalar.activation(out=q[0:40, 0:L],
                                     in_=hq[0:40, t0:t0 + L],
                                     func=mybir.ActivationFunctionType.Identity,
                                     bias=nb_t[:], scale=s_t[:])
                nc.sync.dma_start(out=out_d[0:40, t0:t0 + L],
                                  in_=q[0:40, 0:L])
            nc.sync.dma_start(out=out_d[0:40, SHP:SHP + 4],
                              in_=mn_t[:].bitcast(u8))
            nc.sync.dma_start(out=out_d[0:40, SHP + 4:SHP + 8],
                              in_=mx_t[:].bitcast(u8))
    nc.compile()
    return nc


class _Runner:
    """Caches the jitted PJRT executable for one compiled bass program."""

    def __init__(self, nc):
        import jax
        from jax.sharding import PartitionSpec
        from jax.experimental.shard_map import shard_map
        from concourse import mybir, bass2jax
        from concourse.bass2jax import _bass_exec_p, partition_id_tensor

        bass2jax.install_neuronx_cc_hook()
        self.jax = jax
        self.nc = nc
        self.shard = _shard()

        partition_name = (nc.partition_id_tensor.name
                          if nc.partition_id_tensor else None)
        in_names, out_names, out_avals, out_shapes = [], [], [], []
        for alloc in nc.m.functions[0].allocations:
            if not isinstance(alloc, mybir.MemoryLocationSet):
                continue
            name = alloc.memorylocations[0].name
            if alloc.kind == "ExternalInput":
                if name != partition_name:
                    in_names.append(name)
            elif alloc.kind == "ExternalOutput":
                shape = tuple(alloc.tensor_shape)
                dtype = mybir.dt.np(alloc.dtype)
                out_names.append(name)
                out_avals.append(jax.core.ShapedArray(shape, dtype))
                out_shapes.append((shape, dtype))
        self.in_names = list(in_names)
        self.out_names = list(out_names)
        n_params = len(in_names)
        n_outs = len(out_names)
        all_in = in_names + out_names + (
            [partition_name] if partition_name else [])

        def _body(*args):
            operands = list(args)
            if partition_name is not None:
                operands.append(partition_id_tensor())
            outs = _bass_exec_p.bind(
                *operands, out_avals=tuple(out_avals),
                in_names=tuple(all_in), out_names=tuple(out_names),
                lowering_input_output_aliases=(),
                sim_require_finite=True, sim_require_nnan=True, nc=nc)
            return tuple(outs)

        mesh = self.shard.mesh
        in_specs = (PartitionSpec("core"),) * (n_params + n_outs)
        out_specs = (PartitionSpec("core"),) * n_outs
        # The kernel fully writes its outputs, so the "pre-zeroed output"
        # operands are never read on device: skip donation and reuse one
        # persistent dummy buffer per output across calls.
        self.fn = jax.jit(
            shard_map(_body, mesh=mesh, in_specs=in_specs,
                      out_specs=out_specs, check_rep=False),
            keep_unused=True)
        self.z = tuple(
            jax.device_put(np.zeros((NCORES * s[0], *s[1:]), dt), self.shard)
            for s, dt in out_shapes)

    def __call__(self, arrays):
        args = [arrays[nm] for nm in self.in_names]
        outs = self.fn(*args, *self.z)
        return dict(zip(self.out_names, outs))

    def put(self, a):
        return self.jax.device_put(a, self.shard)


def _nb():
    """Lazily compile the numba helpers (cached on disk after first use)."""
    if "pass1" in _cache:
        return _cache["pass1"], _cache["pass2"]
    import numba

    @numba.njit(cache=True, fastmath=True)
    def pass1(src_e, dst_e, ew):
        # histogram over (group, dst-within-window): edges are emitted
        # dwin-sorted within each group, which turns the shipped d8 stream
        # into sorted runs the transfer tunnel's compressor crushes ~4x
        deg = np.zeros(N, np.float64)
        hist = np.zeros(NG * 128, np.int64)
        for e in range(E + N):
            if e < E:
                s = src_e[e]
                d = dst_e[e]
                deg[d] += ew[e]
            else:
                s = e - E
                d = e - E
                deg[d] += 1.0
            half = 1 if s >= N // 2 else 0
            dl = d % SH
            g = ((d // SH) * 2 + half) * NW + (dl >> 7)
            hist[g * 128 + (dl & 127)] += 1
        return deg, hist

    @numba.njit(cache=True, fastmath=True)
    def pass2(src_e, dst_e, ew, dinv, wp, IDX, D8, NRM8):
        for e in range(E + N):
            if e < E:
                s = src_e[e]
                d = dst_e[e]
                w = ew[e]
            else:
                s = e - E
                d = e - E
                w = np.float32(1.0)
            half = 1 if s >= N // 2 else 0
            cs = s // SH
            cd = d // SH
            dl = d - cd * SH
            g = (cd * 2 + half) * NW + (dl >> 7)
            b = g * 128 + (dl & 127)
            p = wp[b]
            wp[b] = p + 1
            IDX[p] = np.int16((cs - (NCORES // 2) * half) * SHP +
                              (s - cs * SH))
            D8[p] = np.uint8(dl & 127)
            nrm = dinv[s] * w * dinv[d]
            if nrm < 1e-4:
                nrm = 1e-4
            NRM8[p] = np.uint8(-np.log(nrm) * NRM_K + 0.5)

    _cache["pass1"] = pass1
    _cache["pass2"] = pass2
    return pass1, pass2


def _prep_edges(inputs):
    """Edge partitioning, normalization, packing (fused numba passes)."""
    pass1, pass2 = _nb()
    ei = np.asarray(inputs["edge_index"])
    ew = np.asarray(inputs["edge_weight"], np.float32)
    src_e = np.ascontiguousarray(ei[0])
    dst_e = np.ascontiguousarray(ei[1])

    deg, hist = pass1(src_e, dst_e, ew)
    dinv = (1.0 / np.sqrt(np.maximum(deg, 1e-12))).astype(np.float32)

    h2 = hist.reshape(NG, 128)
    gsz = h2.sum(axis=1)
    CC = np.maximum(
        (gsz.reshape(NCORES, 2, NW).max(axis=0) + 127) // 128, 1)  # [2, NW]
    n_chunks = int(CC.sum())
    NIDX = n_chunks * 128

    # write cursor per (group, dwin) bin: core base + padded (half, window)
    # offset + exclusive within-group prefix of smaller dwin values
    pad = (CC.astype(np.int64) * 128).ravel()                # [2*NW]
    off = np.zeros(2 * NW, np.int64)
    off[1:] = np.cumsum(pad)[:-1]
    wp0 = (np.repeat(np.arange(NCORES, dtype=np.int64), 2 * NW) * NIDX +
           np.tile(off, NCORES))
    excl = np.cumsum(h2, axis=1) - h2
    wp = (wp0[:, None] + excl).ravel()

    IDX = np.zeros(NCORES * NIDX, np.int16)
    # pad lanes get dst=255, which never matches the 0..127 iota -> mask 0
    D8 = np.full(NCORES * NIDX, 255, np.uint8)
    NRM = np.zeros(NCORES * NIDX, np.uint8)
    pass2(src_e, dst_e, ew, dinv, wp, IDX, D8, NRM)
    IDX = IDX.reshape(NCORES, NIDX)
    D8 = D8.reshape(NCORES, NIDX)
    NRM = NRM.reshape(NCORES, NIDX)

    idx_g = np.empty((NCORES * 16, NIDX // 16), np.int16)
    d8_g = np.empty((NCORES * 128, n_chunks), np.uint8)
    nrm_g = np.empty((NCORES * 128, n_chunks), np.uint8)
    for c in range(NCORES):
        idx_g[c * 16:(c + 1) * 16] = IDX[c].reshape(-1, 16).T
        d8_g[c * 128:(c + 1) * 128] = D8[c].reshape(-1, 128).T
        nrm_g[c * 128:(c + 1) * 128] = NRM[c].reshape(-1, 128).T
    return CC, {"idx": idx_g, "d8": d8_g, "nrm": nrm_g}


def _prep_weights(inputs):
    """Small replicated tensors: packed weights + folded BN scale/bias."""
    FIN = [128, 128, 128, 64]
    FOUT = [128, 128, 64, 32]
    wpk = np.zeros((128, WPK), np.float16)
    for l in range(4):
        Wl = np.asarray(inputs[f"W{l+1}"], np.float32)
        wpk[:FIN[l], l * 128:l * 128 + FOUT[l]] = Wl.astype(np.float16)
    wpk[:, 512:640] = np.arange(128, dtype=np.float16)[None, :]
    wpk[:32, 640:704] = np.asarray(inputs["lw1"], np.float32).astype(np.float16)
    wpk[:64, 704:744] = np.asarray(inputs["lw2"], np.float32).astype(np.float16)

    sc = np.zeros((128, 14), np.float32)
    g1 = np.asarray(inputs["g1"], np.float32)
    s1 = g1 / np.sqrt(np.asarray(inputs["var1"], np.float32) + BN_EPS)
    sc[:, 0] = s1
    sc[:, 1] = np.asarray(inputs["beta1"], np.float32) - \
        np.asarray(inputs["mu1"], np.float32) * s1
    for l in range(4):
        bn = l + 2
        gl = np.asarray(inputs[f"g{bn}"], np.float32)
        a = gl / np.sqrt(np.asarray(inputs[f"var{bn}"], np.float32) + BN_EPS)
        b = (np.asarray(inputs[f"c{l+1}b"], np.float32) -
             np.asarray(inputs[f"mu{bn}"], np.float32)) * a + \
            np.asarray(inputs[f"beta{bn}"], np.float32)
        sc[:FOUT[l], 2 + 2 * l] = a
        sc[:FOUT[l], 3 + 2 * l] = b
    a6 = np.asarray(inputs["g6"], np.float32) / \
        np.sqrt(np.asarray(inputs["var6"], np.float32) + BN_EPS)
    b6 = (np.asarray(inputs["lb1"], np.float32) -
          np.asarray(inputs["mu6"], np.float32)) * a6 + \
        np.asarray(inputs["beta6"], np.float32)
    sc[:64, 10] = a6
    sc[:64, 11] = b6
    sc[:40, 12] = np.asarray(inputs["lb2"], np.float32)
    sc[:, 13] = np.arange(128, dtype=np.float32)

    scg = np.empty((NCORES * 128, 14), np.float32)
    wpkg = np.empty((NCORES * 128, WPK), np.float16)
    for c in range(NCORES):
        scg[c * 128:(c + 1) * 128] = sc
        wpkg[c * 128:(c + 1) * 128] = wpk
    return {"sc": scg, "wpk": wpkg}


def kernel(**inputs):
    import jax

    # cast + pad x to f8 in quarters, dispatching each upload as soon as
    # it is cast so the wire starts early and streams during the edge prep
    x = np.asarray(inputs["x"], np.float32)
    shard = _shard()
    arrays = {}
    r0 = 0
    for i, nr in enumerate(XSPLIT):
        xq = np.zeros((NCORES * nr, 128), F8)
        hi = min(r0 + nr, SH)
        for c in range(NCORES):
            xq[c * nr:c * nr + hi - r0] = x[c * SH + r0:c * SH + hi]
        arrays[f"x{i}"] = jax.device_put(xq, shard)
        r0 += nr
    for k, v in _prep_weights(inputs).items():
        arrays[k] = jax.device_put(v, shard)

    CC, edict = _prep_edges(inputs)
    key = CC.tobytes()
    if key not in _cache:
        nc = _build_program(CC)
        _cache[key] = _Runner(nc)
    run = _cache[key]
    for k, v in edict.items():
        arrays[k] = run.put(v)
    res = run(arrays)

    # fetch shards in parallel, dequantizing each as it arrives
    from concurrent.futures import ThreadPoolExecutor
    out = np.empty((N, C), np.float32)

    def _fetch(shd):
        blk = np.asarray(shd.data)     # [40, SHP+8] u8
        c = shd.index[0].start // 40
        mn = blk[:, SHP:SHP + 4].copy().view(np.float32)[:, 0]
        mx = blk[:, SHP + 4:SHP + 8].copy().view(np.float32)[:, 0]
        s = (mx - mn) * np.float32(1.0 / 255.0)
        out[c * SH:(c + 1) * SH] = \
            (blk[:, :SH] * s[:, None] + mn[:, None]).T

    with ThreadPoolExecutor(NCORES) as ex:
        list(ex.map(_fetch, res["out"].addressable_shards))
    return out
